# revision 1
# baseline (speedup 1.0000x reference)
"""Allegro-style equivariant GNN edge-network on 8 TRN2 NeuronCores — v2.

Data-parallel over edges (EC=16384/core), 512-edge blocks (32/core).
vs baseline v1 (753us):
  - 1-bank psum tiles (N=512) + deep pools -> cross-block pipelining keeps the
    PE dense (HAM stays at 2.4GHz; v1 ran 94% of the span at 1.2GHz)
  - big fused DVE muls via stride-0 free-dim repeat (x2 rep9 / x1 rep4)
  - DVE reads PSUM operands directly (env via PE-replicated Wenv128 weights)
  - biases folded into weights via ones rows / ACT bias (no Identity+bias ops)
  - t000 env-path folded into a comb0-side matmul term (WG01)
"""

import sys

sys.path.insert(0, "/opt/trn_rl_repo")

import numpy as np
import ml_dtypes

BF = ml_dtypes.bfloat16

import concourse.bass as bass
import concourse.mybir as mybir
from concourse import bacc
from concourse.tile import TileContext
from concourse.bass_utils import run_bass_kernel_spmd

E = 131072
NCORES = 8
EC = E // NCORES
C = 16
S = 64
NB = 8
TE = 16
NL = 2
RMAX = 5.0

N = 512
NBLK = EC // N

F32 = mybir.dt.float32
BF16 = mybir.dt.bfloat16
ACT = mybir.ActivationFunctionType


def _Qnp():
    Q = np.zeros((5, 3, 3))
    s = 1.0 / np.sqrt(2.0)
    Q[0, 0, 1] = Q[0, 1, 0] = s
    Q[1, 1, 2] = Q[1, 2, 1] = s
    Q[2] = np.diag([-1.0, -1.0, 2.0]) / np.sqrt(6.0)
    Q[3, 0, 2] = Q[3, 2, 0] = s
    Q[4] = np.diag([1.0, -1.0, 0.0]) * s
    return Q


_Q = _Qnp()
_An = np.einsum('mij,pjk,qki->mpq', _Q, _Q, _Q)
_A = 0.5 * (_An + _An.transpose(0, 2, 1))


def _fold_weights(inp):
    f = lambda a: np.ascontiguousarray(a, dtype=np.float32)
    W = {}
    s0 = 1.0 / np.sqrt(3.0 * C)
    s1 = 1.0 / np.sqrt(4.0 * C)
    s2 = 1.0 / np.sqrt(4.0 * C)

    We1 = np.asarray(inp["W_e1"], np.float64)
    be1 = np.asarray(inp["b_e1"], np.float64)
    We2 = np.asarray(inp["W_e2"], np.float64)
    be2 = np.asarray(inp["b_e2"], np.float64)
    Wenv_e = np.asarray(inp["W_env_e"], np.float64)
    benv_e = np.asarray(inp["b_env_e"], np.float64)
    Wenv = np.asarray(inp["Wenv"], np.float64)
    benv = np.asarray(inp["benv"], np.float64)
    WM1 = np.asarray(inp["Wm1"], np.float64)
    bM1 = np.asarray(inp["bm1"], np.float64)
    WM2 = np.asarray(inp["Wm2"], np.float64)
    bM2 = np.asarray(inp["bm2"], np.float64)
    wtp = np.asarray(inp["w_tp"], np.float64)
    WL0 = np.asarray(inp["Wlin0"], np.float64)
    WL1 = np.asarray(inp["Wlin1"], np.float64)
    WL2 = np.asarray(inp["Wlin2"], np.float64)

    import ml_dtypes as _mld
    _bias1 = float(np.float32(1.2784645427610783).astype(_mld.bfloat16))
    ones_val = _bias1 / (1.0 + np.exp(-_bias1))   # exact value of comb[64]/mh0[64]

    W["Wpe1"] = f(We1)
    W["be1"] = f(be1.reshape(S, 1))
    W["Wpe2"] = f(np.hstack([We2, np.zeros((S, 1))]))          # (64,65)
    W["be2"] = f(np.vstack([be2.reshape(S, 1), [[_bias1]]]))   # (65,1)

    def tile8(w):
        return np.concatenate([w] * 8, axis=1)

    W["Wenv128_0"] = f(np.vstack([
        tile8(Wenv_e), tile8(benv_e.reshape(1, C)) / ones_val]))
    W["Wenv128_1"] = f(np.vstack([
        tile8(WM2[0] @ Wenv[0]),
        tile8((bM2[0] @ Wenv[0] + benv[0]).reshape(1, C)) / ones_val,
    ]))

    w = wtp[0]
    W0, W1, W2 = WL0[0], WL1[0], WL2[0]

    def n1col(a):
        z = np.zeros((a.shape[0], 80)); z[:, 0:48] = a; return z

    def n0col(a):
        z = np.zeros((a.shape[0], 80)); z[:, 64:80] = a; return z

    w011 = np.zeros((48, 48))
    w101 = np.zeros((48, 48))
    for i in range(3):
        for c in range(C):
            w011[i * C + c, i * C:(i + 1) * C] = w[1][c] * W1[c] * s1
            w101[i * C + c, i * C:(i + 1) * C] = w[3][c] * W1[16 + c] * s1
    w01 = w011 + w101
    w1221 = []
    for j in range(3):
        wj = np.zeros((80, 48))
        for m in range(5):
            for i in range(3):
                for c in range(C):
                    wj[m * C + c, i * C:(i + 1) * C] += _Q[m, i, j] * (
                        w[6][c] * W1[32 + c] + w[8][c] * W1[48 + c]) * s1
        w1221.append(wj)
    w022 = np.zeros((80, 80))
    w202 = np.zeros((80, 80))
    for m in range(5):
        for c in range(C):
            w022[m * C + c, m * C:(m + 1) * C] = w[2][c] * W2[c] * s2
            w202[m * C + c, m * C:(m + 1) * C] = w[7][c] * W2[32 + c] * s2
    w02 = w022 + w202
    w112 = []
    for j in range(3):
        wj = np.zeros((48, 80))
        for i in range(3):
            for m in range(5):
                for c in range(C):
                    wj[i * C + c, m * C:(m + 1) * C] += (
                        _Q[m, i, j] * w[5][c] * W2[16 + c] * s2)
        w112.append(wj)
    w222 = []
    for q in range(5):
        wq = np.zeros((80, 80))
        for p in range(5):
            for m in range(5):
                for c in range(C):
                    wq[p * C + c, m * C:(m + 1) * C] += (
                        _A[m, p, q] * w[10][c] * W2[48 + c] * s2)
        w222.append(wq)

    def wt000(l):
        return (wtp[l][0][:, None] * WL0[l][0:16]) * s0

    def w110f(l):
        z = np.zeros((48, 16))
        for i in range(3):
            z[i * C:(i + 1) * C] = wtp[l][4][:, None] * WL0[l][16:32] * s0
        return z

    def w220f(l):
        z = np.zeros((80, 16))
        for m in range(5):
            z[m * C:(m + 1) * C] = wtp[l][9][:, None] * WL0[l][32:48] * s0
        return z

    # pn01 terms (out 80: n1 @0:48, n0 @64:80)
    W["Wx1_01"] = f(n1col(w01))                        # (48,80)  rhs x1t
    for j in range(3):
        W[f"WP3_01_{j}"] = f(n1col(w1221[j]))          # (80,80)  rhs TG80 g j
    W["WP4d_01"] = f(n0col(w220f(0)))                  # (80,80)  rhs TG80 g3
    W["WP1d_01"] = f(n0col(w110f(0)))                  # (48,80)  rhs TG48 g3
    W["WG01"] = f(np.vstack([
        n0col(Wenv_e @ wt000(0)),
        n0col((benv_e @ wt000(0)).reshape(1, 16)) / ones_val,
    ]))                                                # (65,80)  rhs comb0[0:65]

    # pn2 terms (out 80)
    W["Wx2_2"] = f(w02)                                # (80,80)  rhs x2t
    for j in range(3):
        W[f"WP1_2_{j}"] = f(w112[j])                   # (48,80)  rhs TG48 g j
    for q in range(5):
        W[f"WP4_2_{q}"] = f(w222[q])                   # (80,80)  rhs TG80 g 4+q

    W["Wm1s"] = f(np.hstack([WM1[0][0:64], np.zeros((S, 1))]))  # (64,65)
    W["bm1_0"] = f(np.vstack([bM1[0].reshape(S, 1), [[_bias1]]]))
    Wm1i = np.zeros((80, 65)); Wm1i[64:80, 0:64] = WM1[0][64:80]
    W["Wm1i"] = f(Wm1i)                                # (80,65)  rhs NDC
    W["Wm2"] = f(WM2[0])
    W["bm2_0"] = f(bM2[0].reshape(S, 1))

    W["WUBa"] = f(w220f(1))                            # (80,16)  rhs TUAt
    WUBX = np.zeros((80, 16))
    WUBX[0:48] = w110f(1)
    WUBX[64:80] = wt000(1)
    W["WUBX"] = f(WUBX)                                # (80,16)  rhs TUXt
    W["Wm1sb"] = f(WM1[1][0:64])
    W["bm1_1"] = f(bM1[1].reshape(S, 1))
    W["Wm1ib"] = f(WM1[1][64:80])                      # (16,64)  rhs n0bt
    W["Wm2b"] = f(WM2[1])
    W["bm2_1"] = f(bM2[1].reshape(S, 1))
    return W


def _pack_weights(W):
    names = list(W.keys())
    offs = {}
    col = 0
    for nm in names:
        k, m = W[nm].shape
        offs[nm] = (k, m, col)
        col += m
    arr = np.zeros((128, col), BF)
    for nm in names:
        k, m, o = offs[nm]
        arr[:k, o:o + m] = W[nm].astype(BF)
    return arr, offs


def _build_nc(woffs, wcols):
    nc = bacc.Bacc()
    h_p = nc.declare_dram_parameter("h", [24, EC], BF16, isOutput=False)
    g_p = nc.declare_dram_parameter("geom", [9, EC], BF16, isOutput=False)
    wpack_p = nc.declare_dram_parameter("wpack", [128, wcols], BF16, isOutput=False)
    out_p = nc.declare_dram_parameter("out", [NL, S, EC], BF16, isOutput=True)

    h_ap = h_p[:]
    g_ap = g_p[:]
    out_ap = out_p[:]

    def gsrc(offset, pattern):
        return bass.AP(tensor=g_ap.tensor, offset=offset, ap=pattern)

    def rep(tile_ap, k, n):
        return bass.AP(tensor=tile_ap.tensor, offset=tile_ap.offset,
                       ap=[[tile_ap.ap[0][0], tile_ap.ap[0][1]], [0, k], [1, n]])

    with TileContext(nc) as tc:
        with (
            tc.tile_pool(name="const", bufs=1) as constp,
            tc.tile_pool(name="work", bufs=4) as work,
            tc.tile_pool(name="psum", bufs=4, space="PSUM") as psumA,
            tc.tile_pool(name="psumw", bufs=4, space="PSUM") as psumB,
        ):
            wpack = constp.tile([128, wcols], BF16, name="wpack", tag="wpack")
            nc.sync.dma_start(out=wpack, in_=wpack_p[:])

            class _WT:
                def __getitem__(self, nm):
                    k, m, o = woffs[nm]
                    return wpack[:k, o:o + m]

            wt = _WT()

            def ps(nm, parts):
                if parts <= 64:
                    return psumA.tile([80, N], F32, name=nm, tag="psA")
                return psumB.tile([128, N], F32, name=nm, tag="psB")

            # PE warm-up burst once weights land
            warm = psumA.tile([80, N], F32, name="warm", tag="psA")
            nc.tensor.matmul(warm[:1, :1], wpack[:1, :1], wpack[:1, :1],
                             start=True, stop=True)
            for _ in range(10):
                nc.tensor.matmul(warm[:64, :N], wpack[:128, :64],
                                 wpack[:128, 256:256 + N], start=True, stop=True)

            def head(blk):
                o = blk * N
                hT = work.tile([24, N], BF16, name="hT", tag="hT")
                nc.sync.dma_start(out=hT, in_=h_ap[:, slice(o, o + N)])

                GEO80 = work.tile([80, 9 * N], BF16, name="GEO80", tag="GEO80")
                nc.gpsimd.dma_start(
                    out=GEO80[:, 0:3 * N],
                    in_=gsrc(5 * EC + o, [[0, 80], [EC, 3], [1, N]]))
                nc.gpsimd.dma_start(
                    out=GEO80[:, 3 * N:4 * N],
                    in_=gsrc(o, [[EC, 5], [0, 16], [1, N]]))
                nc.gpsimd.dma_start(
                    out=GEO80[:, 4 * N:9 * N],
                    in_=gsrc(o, [[0, 80], [EC, 5], [1, N]]))

                GEO48 = work.tile([48, 4 * N], BF16, name="GEO48", tag="GEO48")
                nc.sync.dma_start(
                    out=GEO48[:, 0:3 * N],
                    in_=gsrc(5 * EC + o, [[0, 48], [EC, 3], [1, N]]))
                nc.sync.dma_start(
                    out=GEO48[:, 3 * N:4 * N],
                    in_=gsrc(5 * EC + o, [[EC, 3], [0, 16], [1, N]]))

                GN = work.tile([80, N], BF16, name="GN", tag="GN")
                nc.gpsimd.dma_start(
                    out=GN[0:48], in_=gsrc(5 * EC + o, [[EC, 3], [0, 16], [1, N]]))
                nc.gpsimd.dma_start(
                    out=GN[48:80], in_=gsrc(8 * EC + o, [[0, 32], [1, N]]))

                pe1 = ps("pe1", 64)
                nc.tensor.matmul(pe1[:64], wt["Wpe1"], hT, start=True, stop=True)
                sb1 = work.tile([64, N], BF16, name="sb1", tag="sb1")
                nc.scalar.activation(sb1, pe1[:64], ACT.Silu, bias=wt["be1"])
                pe2 = ps("pe2", 64)
                nc.tensor.matmul(pe2[:65], wt["Wpe2"], sb1, start=True, stop=True)
                comb0 = work.tile([65, N], BF16, name="comb0", tag="comb0")
                nc.scalar.activation(comb0[0:65], pe2[:65], ACT.Silu, bias=wt["be2"])

                env0 = ps("env0", 128)
                nc.tensor.matmul(env0[:128], wt["Wenv128_0"], comb0[0:65],
                                 start=True, stop=True)

                gdiag = GEO80[:, 3 * N:4 * N]
                ddiag = GEO48[:, 3 * N:4 * N]
                x2t = work.tile([80, N], BF16, name="x2t", tag="x2t")
                nc.vector.tensor_mul(x2t, env0[0:80], gdiag)
                x1t = work.tile([48, N], BF16, name="x1t", tag="x1t")
                nc.vector.tensor_mul(x1t, env0[64:112], ddiag)
                TG80 = work.tile([80, 9 * N], BF16, name="TG80", tag="TG80")
                nc.vector.tensor_mul(TG80, rep(x2t[:], 9, N), GEO80)
                TG48 = work.tile([48, 4 * N], BF16, name="TG48", tag="TG48")
                nc.vector.tensor_mul(TG48, rep(x1t[:], 4, N), GEO48)

                pn01 = ps("pn01", 80)
                t01 = [("Wx1_01", x1t[:]), ("WP3_01_0", TG80[:, 0:N]),
                       ("WP3_01_1", TG80[:, N:2 * N]), ("WP3_01_2", TG80[:, 2 * N:3 * N]),
                       ("WP4d_01", TG80[:, 3 * N:4 * N]), ("WP1d_01", TG48[:, 3 * N:4 * N]),
                       ("WG01", comb0[0:65])]
                for i, (wn, rhs) in enumerate(t01):
                    nc.tensor.matmul(pn01[:80], wt[wn], rhs,
                                     start=(i == 0), stop=(i == len(t01) - 1))
                pn2 = ps("pn2", 80)
                t2 = [("Wx2_2", x2t[:]), ("WP1_2_0", TG48[:, 0:N]),
                      ("WP1_2_1", TG48[:, N:2 * N]), ("WP1_2_2", TG48[:, 2 * N:3 * N]),
                      ("WP4_2_0", TG80[:, 4 * N:5 * N]), ("WP4_2_1", TG80[:, 5 * N:6 * N]),
                      ("WP4_2_2", TG80[:, 6 * N:7 * N]), ("WP4_2_3", TG80[:, 7 * N:8 * N]),
                      ("WP4_2_4", TG80[:, 8 * N:9 * N])]
                for i, (wn, rhs) in enumerate(t2):
                    nc.tensor.matmul(pn2[:80], wt[wn], rhs,
                                     start=(i == 0), stop=(i == len(t2) - 1))
                NDC = work.tile([80, N], BF16, name="NDC", tag="NDC")
                nc.vector.tensor_mul(NDC, pn01[:80], GN)
                NG = work.tile([80, N], BF16, name="NG", tag="NG")
                nc.vector.tensor_mul(NG, pn2[:80], gdiag)
                return dict(comb0=comb0, NDC=NDC, NG=NG)

            def tail(blk, st):
                o = blk * N
                sl = slice(o, o + N)
                comb0, NDC, NG = st["comb0"], st["NDC"], st["NG"]

                pm1 = ps("pm1", 64)
                nc.tensor.matmul(pm1[:65], wt["Wm1s"], comb0[0:64],
                                 start=True, stop=False)
                nc.tensor.matmul(pm1[:65], wt["Wm1i"], NDC,
                                 start=False, stop=True)
                mh0 = work.tile([65, N], BF16, name="mh0", tag="mh0")
                nc.scalar.activation(mh0[0:65], pm1[:65], ACT.Silu, bias=wt["bm1_0"])
                pm2 = ps("pm2", 64)
                nc.tensor.matmul(pm2[:64], wt["Wm2"], mh0[0:64],
                                 start=True, stop=True)
                comb1 = work.tile([64, N], BF16, name="comb1", tag="comb1")
                nc.scalar.activation(comb1, pm2[:64], ACT.Identity, bias=wt["bm2_0"])
                nc.sync.dma_start(out=out_ap[0, :, sl], in_=comb1)

                env1 = ps("env1", 128)
                nc.tensor.matmul(env1[:128], wt["Wenv128_1"], mh0[0:65],
                                 start=True, stop=True)

                TUAt = work.tile([80, N], BF16, name="TUAt", tag="TUAt")
                nc.vector.tensor_mul(TUAt, NG, env1[0:80])
                TUXt = work.tile([80, N], BF16, name="TUXt", tag="TUXt")
                nc.vector.tensor_mul(TUXt, NDC, env1[0:80])

                pn0b = ps("pn0b", 16)
                nc.tensor.matmul(pn0b[:16], wt["WUBa"], TUAt, start=True, stop=False)
                nc.tensor.matmul(pn0b[:16], wt["WUBX"], TUXt, start=False, stop=True)
                n0bt = work.tile([16, N], BF16, name="n0bt", tag="n0bt")
                nc.scalar.activation(n0bt, pn0b[:16], ACT.Copy)

                pm1b = ps("pm1b", 64)
                nc.tensor.matmul(pm1b[:64], wt["Wm1sb"], comb1,
                                 start=True, stop=False)
                nc.tensor.matmul(pm1b[:64], wt["Wm1ib"], n0bt,
                                 start=False, stop=True)
                mh1 = work.tile([64, N], BF16, name="mh1", tag="mh1")
                nc.scalar.activation(mh1, pm1b[:64], ACT.Silu, bias=wt["bm1_1"])
                pm2b = ps("pm2b", 64)
                nc.tensor.matmul(pm2b[:64], wt["Wm2b"], mh1, start=True, stop=True)
                scal2 = work.tile([64, N], BF16, name="scal2", tag="scal2")
                nc.scalar.activation(scal2, pm2b[:64], ACT.Identity, bias=wt["bm2_1"])
                nc.sync.dma_start(out=out_ap[1, :, sl], in_=scal2)

            sts = [head(0), head(1)]
            warmb = psumA.tile([80, N], F32, name="warmb", tag="psA")
            for _ in range(8):
                nc.tensor.matmul(warmb[:64, :N], wpack[:128, :64],
                                 wpack[:128, 256:256 + N], start=True, stop=True)
            for blk in range(NBLK):
                if blk + 2 < NBLK:
                    sts.append(head(blk + 2))
                tail(blk, sts[blk])
                sts[blk] = None
    nc.finalize()
    return nc


_NC_CACHE = None


def _host_prep(inputs):
    bond_dist = np.asarray(inputs["bond_dist"], np.float32)
    bond_diff = np.asarray(inputs["bond_diff"], np.float32)
    emb = np.asarray(inputs["emb_table"], np.float32)
    Z = np.asarray(inputs["Z"]).astype(np.int64)
    ei = np.asarray(inputs["edge_index"]).astype(np.int64)

    u = bond_dist / RMAX
    n = np.arange(1, NB + 1, dtype=np.float32)
    radial = (np.sqrt(np.float32(2.0 / RMAX)) *
              np.sin(np.float32(np.pi) * n * u[:, None].astype(np.float32)) /
              bond_dist[:, None])
    cutoff = np.where(u < 1.0, 1.0 - 28.0 * u**6 + 48.0 * u**7 - 21.0 * u**8, 0.0)
    radial = (radial * cutoff[:, None].astype(np.float32)).astype(np.float32)

    d = (bond_diff / (bond_dist[:, None] + np.float32(1e-8))).astype(np.float32)
    y2 = (np.sqrt(np.float32(1.5)) *
          np.einsum('mij,ei,ej->em', _Q.astype(np.float32), d, d)).astype(np.float32)

    te = (emb[Z[ei[:, 0]]] * emb[Z[ei[:, 1]]]).astype(np.float32)

    h = np.ascontiguousarray(np.concatenate([radial, te], axis=1).T.astype(BF))
    ones = np.ones((E, 1), np.float32)
    geom = np.ascontiguousarray(
        np.concatenate([y2, d, ones], axis=1).T.astype(BF))
    W = _fold_weights(inputs)
    return h, geom, W


def make_in_maps(inputs):
    global _NC_CACHE
    h, geom, W = _host_prep(inputs)
    wpack, woffs = _pack_weights(W)
    if _NC_CACHE is None:
        _NC_CACHE = _build_nc(woffs, wpack.shape[1])
    in_maps = []
    for i in range(NCORES):
        sl = slice(i * EC, (i + 1) * EC)
        m = {"h": np.ascontiguousarray(h[:, sl]),
             "geom": np.ascontiguousarray(geom[:, sl]),
             "wpack": wpack}
        in_maps.append(m)
    return in_maps


def kernel(**inputs):
    in_maps = make_in_maps(inputs)
    res = run_bass_kernel_spmd(_NC_CACHE, in_maps, list(range(NCORES))).results
    out = np.concatenate(
        [np.asarray(res[i]["out"]).astype(np.float32).transpose(2, 0, 1)
         for i in range(NCORES)], axis=0)
    return np.ascontiguousarray(out)



# revision 17
# speedup vs baseline: 1.4988x; 1.4988x over previous
"""Allegro-style equivariant GNN edge-network on 8 TRN2 NeuronCores — v3.

Key identity: at layer 0, x0/x1/x2 = (y-irrep) * env[c], so the full O3
tensor-product + equivariant-linear stack collapses to per-edge *scalar*
geometry features (s2=|d|^2, s4=|d|^4, vd=v.d, ay=a.y2 — host precomputed)
combined with 16x16 folded weight matmuls on env:
  n1[i,c] = d_i*(A1@env)[c] + v_i*(A2@env)[c]
  n2[m,c] = y2_m*(B1@env)[c] + a_m*(B2@env)[c]
  n0[c]   = (C1@env + s2*C2@env + s4*C3@env)[c]
Layer-1 only needs the invariant channel, which contracts these with d/y2
again, yielding s2/s4/vd/ay scalars. Per 512-edge block the whole network
is 9 matmuls, 4 silu activations and ~8 small DVE ops.

Data-parallel over edges: EC=16384/core, 512-edge blocks (32/core).
"""

import sys

sys.path.insert(0, "/opt/trn_rl_repo")

import numpy as np
import ml_dtypes

BF = ml_dtypes.bfloat16

import concourse.bass as bass
import concourse.mybir as mybir
from concourse import bacc
from concourse.tile import TileContext
from concourse.bass_utils import run_bass_kernel_spmd

E = 131072
NCORES = 8
EC = E // NCORES
C = 16
S = 64
NB = 8
TE = 16
NL = 2
RMAX = 5.0

N = 512
NBLK = EC // N

F32 = mybir.dt.float32
BF16 = mybir.dt.bfloat16
ACT = mybir.ActivationFunctionType


def _Qnp():
    Q = np.zeros((5, 3, 3))
    s = 1.0 / np.sqrt(2.0)
    Q[0, 0, 1] = Q[0, 1, 0] = s
    Q[1, 1, 2] = Q[1, 2, 1] = s
    Q[2] = np.diag([-1.0, -1.0, 2.0]) / np.sqrt(6.0)
    Q[3, 0, 2] = Q[3, 2, 0] = s
    Q[4] = np.diag([1.0, -1.0, 0.0]) * s
    return Q


_Q = _Qnp()
_An = np.einsum('mij,pjk,qki->mpq', _Q, _Q, _Q)
_A = 0.5 * (_An + _An.transpose(0, 2, 1))


def _fold_weights(inp):
    f = lambda a: np.ascontiguousarray(a, dtype=np.float32)
    W = {}
    s0 = 1.0 / np.sqrt(3.0 * C)
    s1 = 1.0 / np.sqrt(4.0 * C)
    s2c = 1.0 / np.sqrt(4.0 * C)

    We1 = np.asarray(inp["W_e1"], np.float64)
    be1 = np.asarray(inp["b_e1"], np.float64)
    We2 = np.asarray(inp["W_e2"], np.float64)
    be2 = np.asarray(inp["b_e2"], np.float64)
    Wenv_e = np.asarray(inp["W_env_e"], np.float64)
    benv_e = np.asarray(inp["b_env_e"], np.float64)
    Wenv = np.asarray(inp["Wenv"], np.float64)
    benv = np.asarray(inp["benv"], np.float64)
    WM1 = np.asarray(inp["Wm1"], np.float64)
    bM1 = np.asarray(inp["bm1"], np.float64)
    WM2 = np.asarray(inp["Wm2"], np.float64)
    bM2 = np.asarray(inp["bm2"], np.float64)
    wtp = np.asarray(inp["w_tp"], np.float64)
    WL0 = np.asarray(inp["Wlin0"], np.float64)
    WL1 = np.asarray(inp["Wlin1"], np.float64)
    WL2 = np.asarray(inp["Wlin2"], np.float64)

    _bias1 = float(np.float32(1.2784645427610783).astype(BF))
    ones_val = _bias1 / (1.0 + np.exp(-_bias1))  # exact bf16-rounded silu(bias1)

    # edge MLP
    W["Wpe1"] = f(We1)                                        # (24,64)
    W["be1"] = f(be1.reshape(S, 1))
    W["Wpe2"] = f(np.hstack([We2, np.zeros((S, 1))]))         # (64,65)
    W["be2"] = f(np.vstack([be2.reshape(S, 1), [[_bias1]]]))  # (65,1)

    # env0: 3 replicated env copies, bias via ones-row (row 64)
    W["Wenv0"] = f(np.vstack([
        np.hstack([Wenv_e] * 3),
        np.hstack([benv_e.reshape(1, C)] * 3) / ones_val,
    ]))                                                       # (65,48)

    # collapsed layer-0 TP: WA maps EQ48 -> [P1 P2 P3 P4 n0]
    w = wtp[0]
    A1 = s1 * (w[1][:, None] * WL1[0][0:16] + w[3][:, None] * WL1[0][16:32])
    A2 = s1 * (w[6][:, None] * WL1[0][32:48] + w[8][:, None] * WL1[0][48:64])
    B1 = s2c * (w[2][:, None] * WL2[0][0:16]
                + (w[5] / np.sqrt(1.5))[:, None] * WL2[0][16:32]
                + w[7][:, None] * WL2[0][32:48])
    B2 = s2c * (w[10][:, None] * WL2[0][48:64])
    C1 = s0 * (w[0][:, None] * WL0[0][0:16])
    C2 = s0 * (w[4][:, None] * WL0[0][16:32])
    C3 = s0 * (w[9][:, None] * WL0[0][32:48])
    WA = np.zeros((48, 80))
    WA[0:16, 0:16] = A1
    WA[0:16, 16:32] = A2
    WA[0:16, 32:48] = B1
    WA[0:16, 48:64] = B2
    WA[0:16, 64:80] = C1
    WA[16:32, 64:80] = C2
    WA[32:48, 64:80] = C3
    W["WA"] = f(WA)                                           # (48,80)

    # pm1: rhs = PMT[0:112] = [comb0(0:65 incl ones@64); stale(65:96); n0(96:112)]
    Wpm1 = np.zeros((112, 65))
    Wpm1[0:64, 0:64] = WM1[0][0:64]
    Wpm1[96:112, 0:64] = WM1[0][64:80]
    W["Wpm1"] = f(Wpm1)                                       # (112,65)
    W["bm1a"] = f(np.vstack([bM1[0].reshape(S, 1), [[_bias1]]]))

    # pm2 + env1 fused: rhs = mh0[0:65]; out rows 0:64 = pm2 (scal1 pre-bias),
    # rows 64:128 = env1 x4 copies (bias via ones-row 64 of mh0)
    Wpm2e = np.zeros((65, 128))
    Wpm2e[0:64, 0:64] = WM2[0]
    Wpm2e[0:64, 64:128] = np.hstack([WM2[0] @ Wenv[0]] * 4)
    Wpm2e[64, 64:128] = np.hstack([(bM2[0] @ Wenv[0] + benv[0]).reshape(1, C)] * 4)[0] / ones_val
    W["Wpm2e"] = f(Wpm2e)                                     # (65,128)
    W["bm2a"] = f(bM2[0].reshape(S, 1))

    # pm1b main: rhs = CT = [comb1(0:64); V64(64:128)]
    wb = wtp[1]
    F110 = s0 * (wb[4][:, None] * WL0[1][16:32]) @ WM1[1][64:80]
    F220 = s0 * (wb[9][:, None] * WL0[1][32:48]) @ WM1[1][64:80]
    F000 = s0 * (wb[0][:, None] * WL0[1][0:16]) @ WM1[1][64:80]
    WB = np.zeros((128, 64))
    WB[0:64] = WM1[1][0:64]
    WB[64:80] = F110
    WB[80:96] = F110
    WB[96:112] = F220
    WB[112:128] = F220
    W["WB"] = f(WB)                                           # (128,64)
    W["WV"] = f(F000)                                         # (16,64)
    W["bm1b"] = f(bM1[1].reshape(S, 1))
    W["Wpm2b"] = f(WM2[1])                                    # (64,64)
    W["bm2b"] = f(bM2[1].reshape(S, 1))
    return W


def _pack_weights(W):
    names = list(W.keys())
    offs = {}
    col = 0
    for nm in names:
        k, m = W[nm].shape
        offs[nm] = (k, m, col)
        col += m
    arr = np.zeros((128, col), BF)
    for nm in names:
        k, m, o = offs[nm]
        arr[:k, o:o + m] = W[nm].astype(BF)
    return arr, offs


def _build_nc(woffs, wcols):
    nc = bacc.Bacc()
    h_p = nc.declare_dram_parameter("h", [24, EC], BF16, isOutput=False)
    g_p = nc.declare_dram_parameter("geo", [112, EC], BF16, isOutput=False)
    wpack_p = nc.declare_dram_parameter("wpack", [128, wcols], BF16, isOutput=False)
    b32_p = nc.declare_dram_parameter("b32", [64, 2], F32, isOutput=False)
    out_p = nc.declare_dram_parameter("out", [NL, S, EC], BF16, isOutput=True)

    h_ap = h_p[:]
    g_ap = g_p[:]
    out_ap = out_p[:]

    with TileContext(nc) as tc:
        with (
            tc.tile_pool(name="const", bufs=1) as constp,
            tc.tile_pool(name="work", bufs=4) as work,
            tc.tile_pool(name="psA", bufs=6, space="PSUM") as psA,
            tc.tile_pool(name="psB", bufs=2, space="PSUM") as psB,
        ):
            wpack = constp.tile([128, wcols], BF16, name="wpack", tag="wpack")
            nc.sync.dma_start(out=wpack, in_=wpack_p[:])
            b32 = constp.tile([64, 2], F32, name="b32", tag="b32")
            nc.sync.dma_start(out=b32, in_=b32_p[:])

            class _WT:
                def __getitem__(self, nm):
                    k, m, o = woffs[nm]
                    return wpack[:k, o:o + m]

            wt = _WT()

            def psa(nm):
                return psA.tile([80, N], F32, name=nm, tag="psA")

            def psb(nm):
                return psB.tile([128, N], F32, name=nm, tag="psB")

            # PE warm-up burst once weights land (flips HAM to 8/8)
            warm = psa("warm")
            nc.tensor.matmul(warm[:1, :1], wpack[:1, :1], wpack[:1, :1],
                             start=True, stop=True)
            for _ in range(10):
                nc.tensor.matmul(warm[:64, :N], wpack[:128, :64],
                                 wpack[:128, :N], start=True, stop=True)

            def dma_in(blk):
                o = blk * N
                sl = slice(o, o + N)
                hT = work.tile([24, N], BF16, name="hT", tag="hT")
                nc.sync.dma_start(out=hT, in_=h_ap[:, sl])
                GTa = work.tile([48, N], BF16, name="GTa", tag="GTa")
                nc.sync.dma_start(out=GTa, in_=g_ap[0:48, sl])
                GTb = work.tile([64, N], BF16, name="GTb", tag="GTb")
                nc.sync.dma_start(out=GTb, in_=g_ap[48:112, sl])
                return hT, GTa, GTb

            def block(blk, st):
                o = blk * N
                sl = slice(o, o + N)
                hT, GTa, GTb = st

                PMT = work.tile([128, N], BF16, name="PMT", tag="PMT")
                # rows 65:96 are dead weight-pad K-rows for pm1 (zero weights);
                # zero them so no NaN bits can poison the PE accumulate.
                nc.gpsimd.memset(PMT[64:96], 0)

                pe1 = psa("pe1")
                nc.tensor.matmul(pe1[:64], wt["Wpe1"], hT, start=True, stop=True)
                sb1 = work.tile([64, N], BF16, name="sb1", tag="sb1")
                nc.scalar.activation(sb1, pe1[:64], ACT.Silu, bias=wt["be1"])

                pe2 = psa("pe2")
                nc.tensor.matmul(pe2[:65], wt["Wpe2"], sb1, start=True, stop=True)
                nc.scalar.activation(PMT[0:65], pe2[:65], ACT.Silu, bias=wt["be2"])

                env0 = psa("env0")
                nc.tensor.matmul(env0[:48], wt["Wenv0"], PMT[0:65],
                                 start=True, stop=True)

                EQ = work.tile([48, N], BF16, name="EQ", tag="EQ")
                nc.vector.tensor_mul(EQ, env0[0:48], GTa)

                pA = psa("pA")
                nc.tensor.matmul(pA[:80], wt["WA"], EQ, start=True, stop=True)
                nc.scalar.activation(PMT[96:112], pA[64:80], ACT.Copy)

                pm1 = psa("pm1")
                nc.tensor.matmul(pm1[:65], wt["Wpm1"], PMT[0:112],
                                 start=True, stop=True)
                mh0 = work.tile([65, N], BF16, name="mh0", tag="mh0")
                nc.scalar.activation(mh0, pm1[:65], ACT.Silu, bias=wt["bm1a"])

                pmf = psb("pmf")
                nc.tensor.matmul(pmf[:128], wt["Wpm2e"], mh0, start=True, stop=True)

                U64 = work.tile([64, N], BF16, name="U64", tag="U64")
                nc.vector.tensor_mul(U64, pA[0:64], GTb)

                CT = work.tile([128, N], BF16, name="CT", tag="CT")
                nc.vector.tensor_scalar_add(CT[0:64], pmf[0:64], b32[:, 0:1])
                nc.vector.tensor_mul(CT[64:128], U64, pmf[64:128])
                VT = work.tile([16, N], BF16, name="VT", tag="VT")
                nc.vector.tensor_mul(VT, PMT[96:112], pmf[64:80])

                nc.gpsimd.dma_start(out=out_ap[0, :, sl], in_=CT[0:64])

                pm1b = psa("pm1b")
                nc.tensor.matmul(pm1b[:64], wt["WB"], CT, start=True, stop=False)
                nc.tensor.matmul(pm1b[:64], wt["WV"], VT, start=False, stop=True)
                mh1 = work.tile([64, N], BF16, name="mh1", tag="mh1")
                nc.scalar.activation(mh1, pm1b[:64], ACT.Silu, bias=wt["bm1b"])

                pm2b = psa("pm2b")
                nc.tensor.matmul(pm2b[:64], wt["Wpm2b"], mh1, start=True, stop=True)
                ST = work.tile([64, N], BF16, name="ST", tag="ST")
                nc.vector.tensor_scalar_add(ST, pm2b[:64], b32[:, 1:2])
                nc.gpsimd.dma_start(out=out_ap[1, :, sl], in_=ST)

            pf = min(3, NBLK)
            sts = [dma_in(b) for b in range(pf)]
            for blk in range(NBLK):
                if blk + pf < NBLK:
                    sts.append(dma_in(blk + pf))
                block(blk, sts[blk])
                sts[blk] = None
    nc.finalize()
    return nc


_NC_CACHE = None


def _host_prep(inputs):
    bond_dist = np.asarray(inputs["bond_dist"], np.float32)
    bond_diff = np.asarray(inputs["bond_diff"], np.float32)
    emb = np.asarray(inputs["emb_table"], np.float32)
    Z = np.asarray(inputs["Z"]).astype(np.int64)
    ei = np.asarray(inputs["edge_index"]).astype(np.int64)

    u = bond_dist / RMAX
    n = np.arange(1, NB + 1, dtype=np.float32)
    radial = (np.sqrt(np.float32(2.0 / RMAX)) *
              np.sin(np.float32(np.pi) * n * u[:, None].astype(np.float32)) /
              bond_dist[:, None])
    cutoff = np.where(u < 1.0, 1.0 - 28.0 * u**6 + 48.0 * u**7 - 21.0 * u**8, 0.0)
    radial = (radial * cutoff[:, None].astype(np.float32)).astype(np.float32)

    d = (bond_diff / (bond_dist[:, None] + np.float32(1e-8))).astype(np.float32)
    Qf = _Q.astype(np.float32)
    y2 = (np.sqrt(np.float32(1.5)) *
          np.einsum('mij,ei,ej->em', Qf, d, d)).astype(np.float32)
    ss = np.einsum('ei,ei->e', d, d)
    s4 = np.einsum('em,em->e', y2, y2)
    v = np.einsum('em,mij,ej->ei', y2, Qf, d)
    vd = np.einsum('ei,ei->e', v, d)
    a = np.einsum('mpq,ep,eq->em', _A.astype(np.float32), y2, y2)
    ay = np.einsum('em,em->e', a, y2)

    te = (emb[Z[ei[:, 0]]] * emb[Z[ei[:, 1]]]).astype(np.float32)
    h = np.ascontiguousarray(np.concatenate([radial, te], axis=1).T.astype(BF))

    geo = np.empty((112, E), np.float32)
    geo[0:16] = 1.0
    geo[16:32] = ss
    geo[32:48] = s4
    geo[48:64] = ss
    geo[64:80] = vd
    geo[80:96] = s4
    geo[96:112] = ay
    geo = np.ascontiguousarray(geo.astype(BF))

    W = _fold_weights(inputs)
    b32 = np.zeros((64, 2), np.float32)
    b32[:, 0] = W["bm2a"][:, 0]
    b32[:, 1] = W["bm2b"][:, 0]
    return h, geo, W, b32


def make_in_maps(inputs):
    global _NC_CACHE
    h, geo, W, b32 = _host_prep(inputs)
    wpack, woffs = _pack_weights(W)
    if _NC_CACHE is None:
        _NC_CACHE = _build_nc(woffs, wpack.shape[1])
    in_maps = []
    for i in range(NCORES):
        sl = slice(i * EC, (i + 1) * EC)
        m = {"h": np.ascontiguousarray(h[:, sl]),
             "geo": np.ascontiguousarray(geo[:, sl]),
             "wpack": wpack, "b32": b32}
        in_maps.append(m)
    return in_maps


def kernel(**inputs):
    in_maps = make_in_maps(inputs)
    res = run_bass_kernel_spmd(_NC_CACHE, in_maps, list(range(NCORES))).results
    out = np.concatenate(
        [np.asarray(res[i]["out"]).astype(np.float32).transpose(2, 0, 1)
         for i in range(NCORES)], axis=0)
    return np.ascontiguousarray(out)


# revision 27
# speedup vs baseline: 2.5978x; 1.7333x over previous
"""Allegro-style equivariant GNN edge-network on 8 TRN2 NeuronCores — v4.

Key identity: at layer 0, x0/x1/x2 = (y-irrep) * env[c], so the full O3
tensor-product + equivariant-linear stack collapses to per-edge *scalar*
geometry features (s2=|d|^2, s4=|d|^4, vd=v.d, ay=a.y2 — host precomputed)
combined with 16x16 folded weight matmuls on env:
  n1[i,c] = d_i*(A1@env)[c] + v_i*(A2@env)[c]
  n2[m,c] = y2_m*(B1@env)[c] + a_m*(B2@env)[c]
  n0[c]   = (C1@env + s2*C2@env + s4*C3@env)[c]
Layer-1 needs only the invariant channel -> 8 matmuls, 4 silus, 5 DVE ops,
3 gpsimd ops per 512-edge block.

v4: 5-stage software pipeline across blocks (interleaved emission keeps all
engine queues fed), one PSUM bank per tile kind (pe1/pm2b share a bank),
single fused out-DMA, no memsets (pad rows produced as silu(-20)~=0).
"""

import sys

sys.path.insert(0, "/opt/trn_rl_repo")

import numpy as np
import ml_dtypes

BF = ml_dtypes.bfloat16

import concourse.bass as bass
import concourse.mybir as mybir
from concourse import bacc
from concourse.tile import TileContext
from concourse.bass_utils import run_bass_kernel_spmd

E = 131072
NCORES = 8
EC = E // NCORES
C = 16
S = 64
NB = 8
TE = 16
NL = 2
RMAX = 5.0

N = 512
NBLK = EC // N

F32 = mybir.dt.float32
BF16 = mybir.dt.bfloat16
ACT = mybir.ActivationFunctionType


def _Qnp():
    Q = np.zeros((5, 3, 3))
    s = 1.0 / np.sqrt(2.0)
    Q[0, 0, 1] = Q[0, 1, 0] = s
    Q[1, 1, 2] = Q[1, 2, 1] = s
    Q[2] = np.diag([-1.0, -1.0, 2.0]) / np.sqrt(6.0)
    Q[3, 0, 2] = Q[3, 2, 0] = s
    Q[4] = np.diag([1.0, -1.0, 0.0]) * s
    return Q


_Q = _Qnp()
_An = np.einsum('mij,pjk,qki->mpq', _Q, _Q, _Q)
_A = 0.5 * (_An + _An.transpose(0, 2, 1))


def _fold_weights(inp):
    f = lambda a: np.ascontiguousarray(a, dtype=np.float32)
    W = {}
    s0 = 1.0 / np.sqrt(3.0 * C)
    s1 = 1.0 / np.sqrt(4.0 * C)
    s2c = 1.0 / np.sqrt(4.0 * C)

    We1 = np.asarray(inp["W_e1"], np.float64)
    be1 = np.asarray(inp["b_e1"], np.float64)
    We2 = np.asarray(inp["W_e2"], np.float64)
    be2 = np.asarray(inp["b_e2"], np.float64)
    Wenv_e = np.asarray(inp["W_env_e"], np.float64)
    benv_e = np.asarray(inp["b_env_e"], np.float64)
    Wenv = np.asarray(inp["Wenv"], np.float64)
    benv = np.asarray(inp["benv"], np.float64)
    WM1 = np.asarray(inp["Wm1"], np.float64)
    bM1 = np.asarray(inp["bm1"], np.float64)
    WM2 = np.asarray(inp["Wm2"], np.float64)
    bM2 = np.asarray(inp["bm2"], np.float64)
    wtp = np.asarray(inp["w_tp"], np.float64)
    WL0 = np.asarray(inp["Wlin0"], np.float64)
    WL1 = np.asarray(inp["Wlin1"], np.float64)
    WL2 = np.asarray(inp["Wlin2"], np.float64)

    _bias1 = float(np.float32(1.2784645427610783).astype(BF))
    ones_val = _bias1 / (1.0 + np.exp(-_bias1))  # exact bf16-rounded silu(bias1)

    # edge MLP
    W["Wpe1"] = f(We1)                                        # (24,64)
    W["be1"] = f(be1.reshape(S, 1))
    # M2 out M=96: col 64 = ones-row feed (0 + bias1), cols 65:96 dead pad
    # driven to silu(-20)~=0 so pm1's zero-weight K-rows read defined data.
    W["Wpe2"] = f(np.hstack([We2, np.zeros((S, 32))]))        # (64,96)
    W["be2"] = f(np.vstack([be2.reshape(S, 1), [[_bias1]],
                            np.full((31, 1), -20.0)]))        # (96,1)

    # env0: 3 replicated env copies, bias via ones-row (row 64)
    W["Wenv0"] = f(np.vstack([
        np.hstack([Wenv_e] * 3),
        np.hstack([benv_e.reshape(1, C)] * 3) / ones_val,
    ]))                                                       # (65,48)

    # collapsed layer-0 TP: WA maps EQ48 -> [P1 P3 P2 P4 n0]
    w = wtp[0]
    A1 = s1 * (w[1][:, None] * WL1[0][0:16] + w[3][:, None] * WL1[0][16:32])
    A2 = s1 * (w[6][:, None] * WL1[0][32:48] + w[8][:, None] * WL1[0][48:64])
    B1 = s2c * (w[2][:, None] * WL2[0][0:16]
                + (w[5] / np.sqrt(1.5))[:, None] * WL2[0][16:32]
                + w[7][:, None] * WL2[0][32:48])
    B2 = s2c * (w[10][:, None] * WL2[0][48:64])
    C1 = s0 * (w[0][:, None] * WL0[0][0:16])
    C2 = s0 * (w[4][:, None] * WL0[0][16:32])
    C3 = s0 * (w[9][:, None] * WL0[0][32:48])
    WA = np.zeros((48, 80))
    WA[0:16, 0:16] = A1
    WA[0:16, 16:32] = B1
    WA[0:16, 32:48] = A2
    WA[0:16, 48:64] = B2
    WA[0:16, 64:80] = C1
    WA[16:32, 64:80] = C2
    WA[32:48, 64:80] = C3
    W["WA"] = f(WA)                                           # (48,80)

    # pm1: rhs = PMT[0:112] = [comb0 0:64; ones@64; ~0 pad 65:96; n0 96:112]
    Wpm1 = np.zeros((112, 65))
    Wpm1[0:64, 0:64] = WM1[0][0:64]
    Wpm1[96:112, 0:64] = WM1[0][64:80]
    W["Wpm1"] = f(Wpm1)                                       # (112,65)
    W["bm1a"] = f(np.vstack([bM1[0].reshape(S, 1), [[_bias1]]]))

    # pm2 + env1 fused: rhs = mh0[0:65]; out rows 0:64 = pm2 (scal1 pre-bias),
    # rows 64:128 = env1 x4 copies (bias via ones-row 64 of mh0)
    Wpm2e = np.zeros((65, 128))
    Wpm2e[0:64, 0:64] = WM2[0]
    Wpm2e[0:64, 64:128] = np.hstack([WM2[0] @ Wenv[0]] * 4)
    Wpm2e[64, 64:128] = np.hstack(
        [(bM2[0] @ Wenv[0] + benv[0]).reshape(1, C)] * 4)[0] / ones_val
    W["Wpm2e"] = f(Wpm2e)                                     # (65,128)

    # pm1b: rhs = CT = [comb1 0:64; V64 = (s2P1,s4P3,vdP2,ayP4)*env1 64:128]
    # pair-sums (s2P1+vdP2 -> t110, s4P3+ayP4 -> t220) folded into repeated
    # F-blocks; t000 (n0*env1) comes in via the separate VT matmul.
    wb = wtp[1]
    F110 = s0 * (wb[4][:, None] * WL0[1][16:32]) @ WM1[1][64:80]
    F220 = s0 * (wb[9][:, None] * WL0[1][32:48]) @ WM1[1][64:80]
    F000 = s0 * (wb[0][:, None] * WL0[1][0:16]) @ WM1[1][64:80]
    WB = np.zeros((128, 64))
    WB[0:64] = WM1[1][0:64]
    WB[64:80] = F110
    WB[80:96] = F220
    WB[96:112] = F110
    WB[112:128] = F220
    W["WB"] = f(WB)                                           # (128,64)
    W["WV"] = f(F000)                                         # (16,64)
    W["bm1b"] = f(bM1[1].reshape(S, 1))
    W["Wpm2b"] = f(WM2[1])                                    # (64,64)
    W["bm2a"] = f(bM2[0].reshape(S, 1))
    W["bm2b"] = f(bM2[1].reshape(S, 1))
    return W


def _pack_weights(W):
    names = list(W.keys())
    offs = {}
    col = 0
    for nm in names:
        k, m = W[nm].shape
        offs[nm] = (k, m, col)
        col += m
    arr = np.zeros((128, col), BF)
    for nm in names:
        k, m, o = offs[nm]
        arr[:k, o:o + m] = W[nm].astype(BF)
    return arr, offs


def _build_nc(woffs, wcols):
    nc = bacc.Bacc()
    h_p = nc.declare_dram_parameter("h", [24, EC], BF16, isOutput=False)
    g_p = nc.declare_dram_parameter("geo", [128, EC], BF16, isOutput=False)
    wpack_p = nc.declare_dram_parameter("wpack", [128, wcols], BF16, isOutput=False)
    b32_p = nc.declare_dram_parameter("b32", [128, 2], F32, isOutput=False)
    out_p = nc.declare_dram_parameter("out", [NL, S, EC], BF16, isOutput=True)

    h_ap = h_p[:]
    g_ap = g_p[:]
    out_ap = out_p[:]

    PF = 4  # dma prefetch depth (iterations)

    with TileContext(nc) as tc:
        with (
            tc.tile_pool(name="const", bufs=1) as constp,
            tc.tile_pool(name="work", bufs=6) as work,
            tc.tile_pool(name="ps1", bufs=1, space="PSUM") as ps1,
            tc.tile_pool(name="ps2", bufs=2, space="PSUM") as ps2,
        ):
            wpack = constp.tile([128, wcols], BF16, name="wpack", tag="wpack")
            nc.sync.dma_start(out=wpack, in_=wpack_p[:])
            b32 = constp.tile([128, 2], F32, name="b32", tag="b32")
            nc.sync.dma_start(out=b32, in_=b32_p[:])

            class _WT:
                def __getitem__(self, nm):
                    k, m, o = woffs[nm]
                    return wpack[:k, o:o + m]

            wt = _WT()

            # PE warm-up burst once weights land (flips HAM to 8/8).
            # Shares the pmf PSUM bank so total stays at 8 banks.
            warm = ps1.tile([112, N], F32, name="warm", tag="pmf")
            nc.tensor.matmul(warm[:1, :1], wpack[:1, :1], wpack[:1, :1],
                             start=True, stop=True)
            for _ in range(10):
                nc.tensor.matmul(warm[:64, :N], wpack[:128, :64],
                                 wpack[:128, :N], start=True, stop=True)

            st = {}  # per-block live tiles

            def dma_in(b):
                o = b * N
                sl = slice(o, o + N)
                hT = work.tile([24, N], BF16, name="hT", tag="hT")
                nc.sync.dma_start(out=hT, in_=h_ap[:, sl])
                GT = work.tile([128, N], BF16, name="GT", tag="GT")
                nc.sync.dma_start(out=GT, in_=g_ap[:, sl])
                st[b] = {"hT": hT, "GT": GT}

            def s1(b):
                d = st[b]
                # pe1 shares a PSUM bank with pm2b (rows 64:128), see s5
                px = ps1.tile([128, N], F32, name="pe1pm2b", tag="pe1pm2b")
                d["px"] = px
                nc.tensor.matmul(px[0:64], wt["Wpe1"], d["hT"],
                                 start=True, stop=True)
                sb1 = work.tile([64, N], BF16, name="sb1", tag="sb1")
                nc.scalar.activation(sb1, px[0:64], ACT.Silu, bias=wt["be1"])
                pe2 = ps1.tile([96, N], F32, name="pe2", tag="pe2")
                nc.tensor.matmul(pe2[:96], wt["Wpe2"], sb1, start=True, stop=True)
                PMT = work.tile([112, N], BF16, name="PMT", tag="PMT")
                d["PMT"] = PMT
                nc.scalar.activation(PMT[0:96], pe2[:96], ACT.Silu, bias=wt["be2"])

            def s2(b):
                d = st[b]
                PMT = d["PMT"]
                env0 = ps1.tile([48, N], F32, name="env0", tag="env0")
                nc.tensor.matmul(env0[:48], wt["Wenv0"], PMT[0:65],
                                 start=True, stop=True)
                EQ = work.tile([48, N], BF16, name="EQ", tag="EQ")
                nc.vector.tensor_mul(EQ, env0[0:48], d["GT"][0:48])
                pA = ps2.tile([80, N], F32, name="pA", tag="pA")
                d["pA"] = pA
                nc.tensor.matmul(pA[:80], wt["WA"], EQ, start=True, stop=True)
                nc.vector.tensor_copy(PMT[96:112], pA[64:80])
                pm1 = ps1.tile([65, N], F32, name="pm1", tag="pm1")
                nc.tensor.matmul(pm1[:65], wt["Wpm1"], PMT[0:112],
                                 start=True, stop=True)
                mh0 = work.tile([65, N], BF16, name="mh0", tag="mh0")
                d["mh0"] = mh0
                nc.scalar.activation(mh0, pm1[:65], ACT.Silu, bias=wt["bm1a"])

            def s3(b):
                d = st[b]
                pmf = ps1.tile([128, N], F32, name="pmf", tag="pmf")
                d["pmf"] = pmf
                nc.tensor.matmul(pmf[:128], wt["Wpm2e"], d["mh0"],
                                 start=True, stop=True)
                U64 = work.tile([64, N], BF16, name="U64", tag="U64")
                nc.vector.tensor_mul(U64, d["pA"][0:64], d["GT"][64:128])
                CT = work.tile([128, N], BF16, name="CT", tag="CT")
                d["CT"] = CT
                nc.scalar.activation(CT[0:64], pmf[0:64], ACT.Identity,
                                     bias=wt["bm2a"])
                nc.vector.tensor_mul(CT[64:128], U64, pmf[64:128])
                VT = work.tile([16, N], BF16, name="VT", tag="VT")
                d["VT"] = VT
                nc.vector.tensor_mul(VT, d["PMT"][96:112], pmf[64:80])

            def s4(b):
                d = st[b]
                pm1b = ps1.tile([64, N], F32, name="pm1b", tag="pm1b")
                d["pm1b"] = pm1b
                nc.tensor.matmul(pm1b[:64], wt["WB"], d["CT"],
                                 start=True, stop=False)
                nc.tensor.matmul(pm1b[:64], wt["WV"], d["VT"],
                                 start=False, stop=True)
                mh1 = work.tile([64, N], BF16, name="mh1", tag="mh1")
                d["mh1"] = mh1
                nc.scalar.activation(mh1, pm1b[:64], ACT.Silu, bias=wt["bm1b"])

            def s5(b):
                d = st[b]
                px = st[b + 4]["px"] if (b + 4) in st else None
                if px is None:
                    px = ps1.tile([128, N], F32, name="pe1pm2b", tag="pe1pm2b")
                nc.tensor.matmul(px[64:128], wt["Wpm2b"], d["mh1"],
                                 start=True, stop=True)
                CT = d["CT"]
                nc.vector.tensor_scalar_add(CT[64:128], px[64:128], b32[64:128, 1:2])
                o = b * N
                dst = bass.AP(tensor=out_ap.tensor, offset=o,
                              ap=[[EC, 128], [1, N]])
                nc.sync.dma_start(out=dst, in_=CT[0:128])
                del st[b]

            for b in range(min(PF, NBLK)):
                dma_in(b)
            for i in range(NBLK + 4):
                if i < NBLK:
                    s1(i)
                if 0 <= i - 1 < NBLK:
                    s2(i - 1)
                if 0 <= i - 2 < NBLK:
                    s3(i - 2)
                if 0 <= i - 3 < NBLK:
                    s4(i - 3)
                if 0 <= i - 4 < NBLK:
                    s5(i - 4)
                if i + PF < NBLK:
                    dma_in(i + PF)
    nc.finalize()
    return nc


_NC_CACHE = None


def _host_prep(inputs):
    bond_dist = np.asarray(inputs["bond_dist"], np.float32)
    bond_diff = np.asarray(inputs["bond_diff"], np.float32)
    emb = np.asarray(inputs["emb_table"], np.float32)
    Z = np.asarray(inputs["Z"]).astype(np.int64)
    ei = np.asarray(inputs["edge_index"]).astype(np.int64)

    u = bond_dist / RMAX
    n = np.arange(1, NB + 1, dtype=np.float32)
    radial = (np.sqrt(np.float32(2.0 / RMAX)) *
              np.sin(np.float32(np.pi) * n * u[:, None].astype(np.float32)) /
              bond_dist[:, None])
    cutoff = np.where(u < 1.0, 1.0 - 28.0 * u**6 + 48.0 * u**7 - 21.0 * u**8, 0.0)
    radial = (radial * cutoff[:, None].astype(np.float32)).astype(np.float32)

    d = (bond_diff / (bond_dist[:, None] + np.float32(1e-8))).astype(np.float32)
    Qf = _Q.astype(np.float32)
    y2 = (np.sqrt(np.float32(1.5)) *
          np.einsum('mij,ei,ej->em', Qf, d, d)).astype(np.float32)
    ss = np.einsum('ei,ei->e', d, d)
    s4 = np.einsum('em,em->e', y2, y2)
    v = np.einsum('em,mij,ej->ei', y2, Qf, d)
    vd = np.einsum('ei,ei->e', v, d)
    a = np.einsum('mpq,ep,eq->em', _A.astype(np.float32), y2, y2)
    ay = np.einsum('em,em->e', a, y2)

    te = (emb[Z[ei[:, 0]]] * emb[Z[ei[:, 1]]]).astype(np.float32)
    h = np.ascontiguousarray(np.concatenate([radial, te], axis=1).T.astype(BF))

    geo = np.zeros((128, E), np.float32)
    geo[0:16] = 1.0
    geo[16:32] = ss
    geo[32:48] = s4
    # rows 48:64 pad (zeros)
    geo[64:80] = ss
    geo[80:96] = s4
    geo[96:112] = vd
    geo[112:128] = ay
    geo = np.ascontiguousarray(geo.astype(BF))

    W = _fold_weights(inputs)
    b32 = np.zeros((128, 2), np.float32)
    b32[0:64, 0] = W["bm2a"][:, 0]
    b32[0:64, 1] = W["bm2b"][:, 0]
    b32[64:128, 1] = W["bm2b"][:, 0]
    return h, geo, W, b32


def make_in_maps(inputs):
    global _NC_CACHE
    h, geo, W, b32 = _host_prep(inputs)
    wpack, woffs = _pack_weights(W)
    if _NC_CACHE is None:
        _NC_CACHE = _build_nc(woffs, wpack.shape[1])
    in_maps = []
    for i in range(NCORES):
        sl = slice(i * EC, (i + 1) * EC)
        m = {"h": np.ascontiguousarray(h[:, sl]),
             "geo": np.ascontiguousarray(geo[:, sl]),
             "wpack": wpack, "b32": b32}
        in_maps.append(m)
    return in_maps


def kernel(**inputs):
    in_maps = make_in_maps(inputs)
    res = run_bass_kernel_spmd(_NC_CACHE, in_maps, list(range(NCORES))).results
    out = np.concatenate(
        [np.asarray(res[i]["out"]).astype(np.float32).transpose(2, 0, 1)
         for i in range(NCORES)], axis=0)
    return np.ascontiguousarray(out)


# revision 28
# speedup vs baseline: 2.7226x; 1.0480x over previous
"""Allegro-style equivariant GNN edge-network on 8 TRN2 NeuronCores — v4.

Key identity: at layer 0, x0/x1/x2 = (y-irrep) * env[c], so the full O3
tensor-product + equivariant-linear stack collapses to per-edge *scalar*
geometry features (s2=|d|^2, s4=|d|^4, vd=v.d, ay=a.y2 — host precomputed)
combined with 16x16 folded weight matmuls on env:
  n1[i,c] = d_i*(A1@env)[c] + v_i*(A2@env)[c]
  n2[m,c] = y2_m*(B1@env)[c] + a_m*(B2@env)[c]
  n0[c]   = (C1@env + s2*C2@env + s4*C3@env)[c]
Layer-1 needs only the invariant channel -> 8 matmuls, 4 silus, 5 DVE ops,
3 gpsimd ops per 512-edge block.

v4: 5-stage software pipeline across blocks (interleaved emission keeps all
engine queues fed), one PSUM bank per tile kind (pe1/pm2b share a bank),
single fused out-DMA, no memsets (pad rows produced as silu(-20)~=0).
"""

import sys

sys.path.insert(0, "/opt/trn_rl_repo")

import numpy as np
import ml_dtypes

BF = ml_dtypes.bfloat16

import concourse.bass as bass
import concourse.mybir as mybir
from concourse import bacc
from concourse.tile import TileContext
from concourse.bass_utils import run_bass_kernel_spmd

E = 131072
NCORES = 8
EC = E // NCORES
C = 16
S = 64
NB = 8
TE = 16
NL = 2
RMAX = 5.0

N = 512
NBLK = EC // N

F32 = mybir.dt.float32
BF16 = mybir.dt.bfloat16
ACT = mybir.ActivationFunctionType


def _Qnp():
    Q = np.zeros((5, 3, 3))
    s = 1.0 / np.sqrt(2.0)
    Q[0, 0, 1] = Q[0, 1, 0] = s
    Q[1, 1, 2] = Q[1, 2, 1] = s
    Q[2] = np.diag([-1.0, -1.0, 2.0]) / np.sqrt(6.0)
    Q[3, 0, 2] = Q[3, 2, 0] = s
    Q[4] = np.diag([1.0, -1.0, 0.0]) * s
    return Q


_Q = _Qnp()
_An = np.einsum('mij,pjk,qki->mpq', _Q, _Q, _Q)
_A = 0.5 * (_An + _An.transpose(0, 2, 1))


def _fold_weights(inp):
    f = lambda a: np.ascontiguousarray(a, dtype=np.float32)
    W = {}
    s0 = 1.0 / np.sqrt(3.0 * C)
    s1 = 1.0 / np.sqrt(4.0 * C)
    s2c = 1.0 / np.sqrt(4.0 * C)

    We1 = np.asarray(inp["W_e1"], np.float64)
    be1 = np.asarray(inp["b_e1"], np.float64)
    We2 = np.asarray(inp["W_e2"], np.float64)
    be2 = np.asarray(inp["b_e2"], np.float64)
    Wenv_e = np.asarray(inp["W_env_e"], np.float64)
    benv_e = np.asarray(inp["b_env_e"], np.float64)
    Wenv = np.asarray(inp["Wenv"], np.float64)
    benv = np.asarray(inp["benv"], np.float64)
    WM1 = np.asarray(inp["Wm1"], np.float64)
    bM1 = np.asarray(inp["bm1"], np.float64)
    WM2 = np.asarray(inp["Wm2"], np.float64)
    bM2 = np.asarray(inp["bm2"], np.float64)
    wtp = np.asarray(inp["w_tp"], np.float64)
    WL0 = np.asarray(inp["Wlin0"], np.float64)
    WL1 = np.asarray(inp["Wlin1"], np.float64)
    WL2 = np.asarray(inp["Wlin2"], np.float64)

    _bias1 = float(np.float32(1.2784645427610783).astype(BF))
    ones_val = _bias1 / (1.0 + np.exp(-_bias1))  # exact bf16-rounded silu(bias1)

    # edge MLP
    W["Wpe1"] = f(We1)                                        # (24,64)
    W["be1"] = f(be1.reshape(S, 1))
    # M2 out M=96: col 64 = ones-row feed (0 + bias1), cols 65:96 dead pad
    # driven to silu(-20)~=0 so pm1's zero-weight K-rows read defined data.
    W["Wpe2"] = f(np.hstack([We2, np.zeros((S, 32))]))        # (64,96)
    W["be2"] = f(np.vstack([be2.reshape(S, 1), [[_bias1]],
                            np.full((31, 1), -20.0)]))        # (96,1)

    # env0: 3 replicated env copies, bias via ones-row (row 64)
    W["Wenv0"] = f(np.vstack([
        np.hstack([Wenv_e] * 3),
        np.hstack([benv_e.reshape(1, C)] * 3) / ones_val,
    ]))                                                       # (65,48)

    # collapsed layer-0 TP: WA maps EQ48 -> [P1 P3 P2 P4 n0]
    w = wtp[0]
    A1 = s1 * (w[1][:, None] * WL1[0][0:16] + w[3][:, None] * WL1[0][16:32])
    A2 = s1 * (w[6][:, None] * WL1[0][32:48] + w[8][:, None] * WL1[0][48:64])
    B1 = s2c * (w[2][:, None] * WL2[0][0:16]
                + (w[5] / np.sqrt(1.5))[:, None] * WL2[0][16:32]
                + w[7][:, None] * WL2[0][32:48])
    B2 = s2c * (w[10][:, None] * WL2[0][48:64])
    C1 = s0 * (w[0][:, None] * WL0[0][0:16])
    C2 = s0 * (w[4][:, None] * WL0[0][16:32])
    C3 = s0 * (w[9][:, None] * WL0[0][32:48])
    WA = np.zeros((48, 80))
    WA[0:16, 0:16] = A1
    WA[0:16, 16:32] = B1
    WA[0:16, 32:48] = A2
    WA[0:16, 48:64] = B2
    WA[0:16, 64:80] = C1
    WA[16:32, 64:80] = C2
    WA[32:48, 64:80] = C3
    W["WA"] = f(WA)                                           # (48,80)

    # pm1: rhs = PMT[0:112] = [comb0 0:64; ones@64; ~0 pad 65:96; n0 96:112]
    Wpm1 = np.zeros((112, 65))
    Wpm1[0:64, 0:64] = WM1[0][0:64]
    Wpm1[96:112, 0:64] = WM1[0][64:80]
    W["Wpm1"] = f(Wpm1)                                       # (112,65)
    W["bm1a"] = f(np.vstack([bM1[0].reshape(S, 1), [[_bias1]]]))

    # pm2 + env1 fused: rhs = mh0[0:65]; out rows 0:64 = pm2 (scal1 pre-bias),
    # rows 64:128 = env1 x4 copies (bias via ones-row 64 of mh0)
    Wpm2e = np.zeros((65, 128))
    Wpm2e[0:64, 0:64] = WM2[0]
    Wpm2e[0:64, 64:128] = np.hstack([WM2[0] @ Wenv[0]] * 4)
    Wpm2e[64, 64:128] = np.hstack(
        [(bM2[0] @ Wenv[0] + benv[0]).reshape(1, C)] * 4)[0] / ones_val
    W["Wpm2e"] = f(Wpm2e)                                     # (65,128)

    # pm1b: rhs = CT = [comb1 0:64; V64 = (s2P1,s4P3,vdP2,ayP4)*env1 64:128]
    # pair-sums (s2P1+vdP2 -> t110, s4P3+ayP4 -> t220) folded into repeated
    # F-blocks; t000 (n0*env1) comes in via the separate VT matmul.
    wb = wtp[1]
    F110 = s0 * (wb[4][:, None] * WL0[1][16:32]) @ WM1[1][64:80]
    F220 = s0 * (wb[9][:, None] * WL0[1][32:48]) @ WM1[1][64:80]
    F000 = s0 * (wb[0][:, None] * WL0[1][0:16]) @ WM1[1][64:80]
    WB = np.zeros((128, 64))
    WB[0:64] = WM1[1][0:64]
    WB[64:80] = F110
    WB[80:96] = F220
    WB[96:112] = F110
    WB[112:128] = F220
    W["WB"] = f(WB)                                           # (128,64)
    W["WV"] = f(F000)                                         # (16,64)
    W["bm1b"] = f(bM1[1].reshape(S, 1))
    W["Wpm2b"] = f(WM2[1])                                    # (64,64)
    W["bm2a"] = f(bM2[0].reshape(S, 1))
    W["bm2b"] = f(bM2[1].reshape(S, 1))
    return W


def _pack_weights(W):
    names = list(W.keys())
    offs = {}
    col = 0
    for nm in names:
        k, m = W[nm].shape
        offs[nm] = (k, m, col)
        col += m
    arr = np.zeros((128, col), BF)
    for nm in names:
        k, m, o = offs[nm]
        arr[:k, o:o + m] = W[nm].astype(BF)
    return arr, offs


def _build_nc(woffs, wcols):
    nc = bacc.Bacc()
    h_p = nc.declare_dram_parameter("h", [24, EC], BF16, isOutput=False)
    g_p = nc.declare_dram_parameter("geo", [112, EC], BF16, isOutput=False)
    wpack_p = nc.declare_dram_parameter("wpack", [128, wcols], BF16, isOutput=False)
    b32_p = nc.declare_dram_parameter("b32", [128, 2], F32, isOutput=False)
    out_p = nc.declare_dram_parameter("out", [NL, S, EC], BF16, isOutput=True)

    h_ap = h_p[:]
    g_ap = g_p[:]
    out_ap = out_p[:]

    PF = 4  # dma prefetch depth (iterations)

    with TileContext(nc) as tc:
        with (
            tc.tile_pool(name="const", bufs=1) as constp,
            tc.tile_pool(name="work", bufs=6) as work,
            tc.tile_pool(name="ps1", bufs=1, space="PSUM") as ps1,
            tc.tile_pool(name="ps2", bufs=2, space="PSUM") as ps2,
        ):
            wpack = constp.tile([128, wcols], BF16, name="wpack", tag="wpack")
            nc.sync.dma_start(out=wpack, in_=wpack_p[:])
            b32 = constp.tile([128, 2], F32, name="b32", tag="b32")
            nc.sync.dma_start(out=b32, in_=b32_p[:])

            class _WT:
                def __getitem__(self, nm):
                    k, m, o = woffs[nm]
                    return wpack[:k, o:o + m]

            wt = _WT()

            # PE warm-up burst once weights land (flips HAM to 8/8).
            # Shares the pmf PSUM bank so total stays at 8 banks.
            warm = ps2.tile([128, N], F32, name="warm", tag="pmf")
            nc.tensor.matmul(warm[:1, :1], wpack[:1, :1], wpack[:1, :1],
                             start=True, stop=True)
            for _ in range(10):
                nc.tensor.matmul(warm[:64, :N], wpack[:128, :64],
                                 wpack[:128, :N], start=True, stop=True)

            st = {}  # per-block live tiles

            def dma_in(b):
                o = b * N
                sl = slice(o, o + N)
                hT = work.tile([24, N], BF16, name="hT", tag="hT")
                nc.sync.dma_start(out=hT, in_=h_ap[:, sl])
                GT = work.tile([112, N], BF16, name="GT", tag="GT")
                nc.sync.dma_start(out=GT, in_=g_ap[:, sl])
                st[b] = {"hT": hT, "GT": GT}

            def s1(b):
                d = st[b]
                # pe1 shares a PSUM bank with pm2b (rows 64:128), see s5
                px = ps1.tile([128, N], F32, name="pe1pm2b", tag="pe1pm2b")
                d["px"] = px
                nc.tensor.matmul(px[0:64], wt["Wpe1"], d["hT"],
                                 start=True, stop=True)
                sb1 = work.tile([64, N], BF16, name="sb1", tag="sb1")
                nc.scalar.activation(sb1, px[0:64], ACT.Silu, bias=wt["be1"])
                pe2 = ps1.tile([96, N], F32, name="pe2", tag="pe2")
                nc.tensor.matmul(pe2[:96], wt["Wpe2"], sb1, start=True, stop=True)
                PMT = work.tile([112, N], BF16, name="PMT", tag="PMT")
                d["PMT"] = PMT
                nc.scalar.activation(PMT[0:96], pe2[:96], ACT.Silu, bias=wt["be2"])

            def s2(b):
                d = st[b]
                PMT = d["PMT"]
                env0 = ps1.tile([48, N], F32, name="env0", tag="env0")
                nc.tensor.matmul(env0[:48], wt["Wenv0"], PMT[0:65],
                                 start=True, stop=True)
                EQ = work.tile([48, N], BF16, name="EQ", tag="EQ")
                nc.vector.tensor_mul(EQ, env0[0:48], d["GT"][64:112])
                pA = ps1.tile([80, N], F32, name="pA", tag="pA")
                nc.tensor.matmul(pA[:80], wt["WA"], EQ, start=True, stop=True)
                U80 = work.tile([80, N], BF16, name="U80", tag="U80")
                d["U80"] = U80
                nc.vector.tensor_mul(U80, pA[0:80], d["GT"][0:80])
                nc.gpsimd.tensor_copy(PMT[96:112], U80[64:80])
                pm1 = ps1.tile([65, N], F32, name="pm1", tag="pm1")
                nc.tensor.matmul(pm1[:65], wt["Wpm1"], PMT[0:112],
                                 start=True, stop=True)
                mh0 = work.tile([65, N], BF16, name="mh0", tag="mh0")
                d["mh0"] = mh0
                nc.scalar.activation(mh0, pm1[:65], ACT.Silu, bias=wt["bm1a"])

            def s3(b):
                d = st[b]
                pmf = ps2.tile([128, N], F32, name="pmf", tag="pmf")
                d["pmf"] = pmf
                nc.tensor.matmul(pmf[:128], wt["Wpm2e"], d["mh0"],
                                 start=True, stop=True)
                CT = work.tile([128, N], BF16, name="CT", tag="CT")
                d["CT"] = CT
                nc.scalar.activation(CT[0:64], pmf[0:64], ACT.Identity,
                                     bias=wt["bm2a"])
                nc.vector.tensor_mul(CT[64:128], d["U80"][0:64], pmf[64:128])
                VT = work.tile([16, N], BF16, name="VT", tag="VT")
                d["VT"] = VT
                nc.vector.tensor_mul(VT, d["U80"][64:80], pmf[64:80])

            def s4(b):
                d = st[b]
                pm1b = ps1.tile([64, N], F32, name="pm1b", tag="pm1b")
                d["pm1b"] = pm1b
                nc.tensor.matmul(pm1b[:64], wt["WB"], d["CT"],
                                 start=True, stop=False)
                nc.tensor.matmul(pm1b[:64], wt["WV"], d["VT"],
                                 start=False, stop=True)
                mh1 = work.tile([64, N], BF16, name="mh1", tag="mh1")
                d["mh1"] = mh1
                nc.scalar.activation(mh1, pm1b[:64], ACT.Silu, bias=wt["bm1b"])

            def s5(b):
                d = st[b]
                px = st[b + 4]["px"] if (b + 4) in st else None
                if px is None:
                    px = ps1.tile([128, N], F32, name="pe1pm2b", tag="pe1pm2b")
                nc.tensor.matmul(px[64:128], wt["Wpm2b"], d["mh1"],
                                 start=True, stop=True)
                CT = d["CT"]
                nc.vector.tensor_scalar_add(CT[64:128], px[64:128], b32[64:128, 1:2])
                o = b * N
                dst = bass.AP(tensor=out_ap.tensor, offset=o,
                              ap=[[EC, 128], [1, N]])
                nc.sync.dma_start(out=dst, in_=CT[0:128])
                del st[b]

            for b in range(min(PF, NBLK)):
                dma_in(b)
            for i in range(NBLK + 4):
                if i < NBLK:
                    s1(i)
                if 0 <= i - 1 < NBLK:
                    s2(i - 1)
                if 0 <= i - 2 < NBLK:
                    s3(i - 2)
                if 0 <= i - 3 < NBLK:
                    s4(i - 3)
                if 0 <= i - 4 < NBLK:
                    s5(i - 4)
                if i + PF < NBLK:
                    dma_in(i + PF)
    nc.finalize()
    return nc


_NC_CACHE = None


def _host_prep(inputs):
    bond_dist = np.asarray(inputs["bond_dist"], np.float32)
    bond_diff = np.asarray(inputs["bond_diff"], np.float32)
    emb = np.asarray(inputs["emb_table"], np.float32)
    Z = np.asarray(inputs["Z"]).astype(np.int64)
    ei = np.asarray(inputs["edge_index"]).astype(np.int64)

    u = bond_dist / RMAX
    n = np.arange(1, NB + 1, dtype=np.float32)
    radial = (np.sqrt(np.float32(2.0 / RMAX)) *
              np.sin(np.float32(np.pi) * n * u[:, None].astype(np.float32)) /
              bond_dist[:, None])
    cutoff = np.where(u < 1.0, 1.0 - 28.0 * u**6 + 48.0 * u**7 - 21.0 * u**8, 0.0)
    radial = (radial * cutoff[:, None].astype(np.float32)).astype(np.float32)

    d = (bond_diff / (bond_dist[:, None] + np.float32(1e-8))).astype(np.float32)
    Qf = _Q.astype(np.float32)
    y2 = (np.sqrt(np.float32(1.5)) *
          np.einsum('mij,ei,ej->em', Qf, d, d)).astype(np.float32)
    ss = np.einsum('ei,ei->e', d, d)
    s4 = np.einsum('em,em->e', y2, y2)
    v = np.einsum('em,mij,ej->ei', y2, Qf, d)
    vd = np.einsum('ei,ei->e', v, d)
    a = np.einsum('mpq,ep,eq->em', _A.astype(np.float32), y2, y2)
    ay = np.einsum('em,em->e', a, y2)

    te = (emb[Z[ei[:, 0]]] * emb[Z[ei[:, 1]]]).astype(np.float32)
    h = np.ascontiguousarray(np.concatenate([radial, te], axis=1).T.astype(BF))

    # rows 0:80 feed U80 = pA * [s2,s4,vd,ay,1]; rows 64:112 = [1,s2,s4]
    # feed EQ (the ones-section is shared between both windows).
    geo = np.zeros((112, E), np.float32)
    geo[0:16] = ss
    geo[16:32] = s4
    geo[32:48] = vd
    geo[48:64] = ay
    geo[64:80] = 1.0
    geo[80:96] = ss
    geo[96:112] = s4
    geo = np.ascontiguousarray(geo.astype(BF))

    W = _fold_weights(inputs)
    b32 = np.zeros((128, 2), np.float32)
    b32[0:64, 0] = W["bm2a"][:, 0]
    b32[0:64, 1] = W["bm2b"][:, 0]
    b32[64:128, 1] = W["bm2b"][:, 0]
    return h, geo, W, b32


def make_in_maps(inputs):
    global _NC_CACHE
    h, geo, W, b32 = _host_prep(inputs)
    wpack, woffs = _pack_weights(W)
    if _NC_CACHE is None:
        _NC_CACHE = _build_nc(woffs, wpack.shape[1])
    in_maps = []
    for i in range(NCORES):
        sl = slice(i * EC, (i + 1) * EC)
        m = {"h": np.ascontiguousarray(h[:, sl]),
             "geo": np.ascontiguousarray(geo[:, sl]),
             "wpack": wpack, "b32": b32}
        in_maps.append(m)
    return in_maps


def kernel(**inputs):
    in_maps = make_in_maps(inputs)
    res = run_bass_kernel_spmd(_NC_CACHE, in_maps, list(range(NCORES))).results
    out = np.concatenate(
        [np.asarray(res[i]["out"]).astype(np.float32).transpose(2, 0, 1)
         for i in range(NCORES)], axis=0)
    return np.ascontiguousarray(out)


# revision 32
# speedup vs baseline: 3.2644x; 1.1990x over previous
"""Allegro-style equivariant GNN edge-network on 8 TRN2 NeuronCores — v4.

Key identity: at layer 0, x0/x1/x2 = (y-irrep) * env[c], so the full O3
tensor-product + equivariant-linear stack collapses to per-edge *scalar*
geometry features (s2=|d|^2, s4=|d|^4, vd=v.d, ay=a.y2 — host precomputed)
combined with 16x16 folded weight matmuls on env:
  n1[i,c] = d_i*(A1@env)[c] + v_i*(A2@env)[c]
  n2[m,c] = y2_m*(B1@env)[c] + a_m*(B2@env)[c]
  n0[c]   = (C1@env + s2*C2@env + s4*C3@env)[c]
Layer-1 needs only the invariant channel -> 8 matmuls, 4 silus, 5 DVE ops,
3 gpsimd ops per 512-edge block.

v4: 5-stage software pipeline across blocks (interleaved emission keeps all
engine queues fed), one PSUM bank per tile kind (pe1/pm2b share a bank),
single fused out-DMA, no memsets (pad rows produced as silu(-20)~=0).
"""

import sys

sys.path.insert(0, "/opt/trn_rl_repo")

import numpy as np
import ml_dtypes

BF = ml_dtypes.bfloat16

import concourse.bass as bass
import concourse.mybir as mybir
from concourse import bacc
from concourse.tile import TileContext
from concourse.bass_utils import run_bass_kernel_spmd

E = 131072
NCORES = 8
EC = E // NCORES
C = 16
S = 64
NB = 8
TE = 16
NL = 2
RMAX = 5.0

N = 512
NBLK = EC // N

F32 = mybir.dt.float32
BF16 = mybir.dt.bfloat16
ACT = mybir.ActivationFunctionType


def _Qnp():
    Q = np.zeros((5, 3, 3))
    s = 1.0 / np.sqrt(2.0)
    Q[0, 0, 1] = Q[0, 1, 0] = s
    Q[1, 1, 2] = Q[1, 2, 1] = s
    Q[2] = np.diag([-1.0, -1.0, 2.0]) / np.sqrt(6.0)
    Q[3, 0, 2] = Q[3, 2, 0] = s
    Q[4] = np.diag([1.0, -1.0, 0.0]) * s
    return Q


_Q = _Qnp()
_An = np.einsum('mij,pjk,qki->mpq', _Q, _Q, _Q)
_A = 0.5 * (_An + _An.transpose(0, 2, 1))


def _fold_weights(inp):
    f = lambda a: np.ascontiguousarray(a, dtype=np.float32)
    W = {}
    s0 = 1.0 / np.sqrt(3.0 * C)
    s1 = 1.0 / np.sqrt(4.0 * C)
    s2c = 1.0 / np.sqrt(4.0 * C)

    We1 = np.asarray(inp["W_e1"], np.float64)
    be1 = np.asarray(inp["b_e1"], np.float64)
    We2 = np.asarray(inp["W_e2"], np.float64)
    be2 = np.asarray(inp["b_e2"], np.float64)
    Wenv_e = np.asarray(inp["W_env_e"], np.float64)
    benv_e = np.asarray(inp["b_env_e"], np.float64)
    Wenv = np.asarray(inp["Wenv"], np.float64)
    benv = np.asarray(inp["benv"], np.float64)
    WM1 = np.asarray(inp["Wm1"], np.float64)
    bM1 = np.asarray(inp["bm1"], np.float64)
    WM2 = np.asarray(inp["Wm2"], np.float64)
    bM2 = np.asarray(inp["bm2"], np.float64)
    wtp = np.asarray(inp["w_tp"], np.float64)
    WL0 = np.asarray(inp["Wlin0"], np.float64)
    WL1 = np.asarray(inp["Wlin1"], np.float64)
    WL2 = np.asarray(inp["Wlin2"], np.float64)

    _bias1 = float(np.float32(1.2784645427610783).astype(BF))
    ones_val = _bias1 / (1.0 + np.exp(-_bias1))  # exact bf16-rounded silu(bias1)

    # merged edge-MLP matmul: rhs = HS = [hT(i) 0:24; sb1(i-1) 32:96],
    # out = [pre-sb1(i) 0:64; pe2(i-1) 64:128]
    WM12 = np.zeros((96, 128))
    WM12[0:64, 64:128] = We2
    WM12[64:88, 0:64] = We1
    W["WM12"] = f(WM12)                                       # (96,128)
    W["be1"] = f(be1.reshape(S, 1))
    W["be2"] = f(be2.reshape(S, 1))

    # env0: 3 replicated env copies; bias folded into the EQ stt op
    W["Wenv0"] = f(np.hstack([Wenv_e] * 3))                   # (64,48)

    # collapsed layer-0 TP: WA maps EQ48 -> [P1 P3 P2 P4 n0]
    w = wtp[0]
    A1 = s1 * (w[1][:, None] * WL1[0][0:16] + w[3][:, None] * WL1[0][16:32])
    A2 = s1 * (w[6][:, None] * WL1[0][32:48] + w[8][:, None] * WL1[0][48:64])
    B1 = s2c * (w[2][:, None] * WL2[0][0:16]
                + (w[5] / np.sqrt(1.5))[:, None] * WL2[0][16:32]
                + w[7][:, None] * WL2[0][32:48])
    B2 = s2c * (w[10][:, None] * WL2[0][48:64])
    C1 = s0 * (w[0][:, None] * WL0[0][0:16])
    C2 = s0 * (w[4][:, None] * WL0[0][16:32])
    C3 = s0 * (w[9][:, None] * WL0[0][32:48])
    WA = np.zeros((48, 80))
    WA[0:16, 0:16] = A1
    WA[0:16, 16:32] = B1
    WA[0:16, 32:48] = A2
    WA[0:16, 48:64] = B2
    WA[0:16, 64:80] = C1
    WA[16:32, 64:80] = C2
    WA[32:48, 64:80] = C3
    W["WA"] = f(WA)                                           # (48,80)

    # pm1: rhs = PMT[0:80] = [comb0 0:64; n0 64:80]
    Wpm1 = np.zeros((80, 65))
    Wpm1[0:64, 0:64] = WM1[0][0:64]
    Wpm1[64:80, 0:64] = WM1[0][64:80]
    W["Wpm1"] = f(Wpm1)                                       # (80,65)
    W["bm1a"] = f(np.vstack([bM1[0].reshape(S, 1), [[_bias1]]]))

    # pm2 + env1 fused: rhs = mh0[0:65]; out rows 0:64 = pm2 (scal1 pre-bias),
    # rows 64:128 = env1 x4 copies (bias via ones-row 64 of mh0)
    Wpm2e = np.zeros((65, 128))
    Wpm2e[0:64, 0:64] = WM2[0]
    Wpm2e[0:64, 64:128] = np.hstack([WM2[0] @ Wenv[0]] * 4)
    Wpm2e[64, 64:128] = np.hstack(
        [(bM2[0] @ Wenv[0] + benv[0]).reshape(1, C)] * 4)[0] / ones_val
    W["Wpm2e"] = f(Wpm2e)                                     # (65,128)

    # pm1b: rhs = CT = [comb1 0:64; V64 = (s2P1,s4P3,vdP2,ayP4)*env1 64:128]
    # pair-sums (s2P1+vdP2 -> t110, s4P3+ayP4 -> t220) folded into repeated
    # F-blocks; t000 (n0*env1) comes in via the separate VT matmul.
    wb = wtp[1]
    F110 = s0 * (wb[4][:, None] * WL0[1][16:32]) @ WM1[1][64:80]
    F220 = s0 * (wb[9][:, None] * WL0[1][32:48]) @ WM1[1][64:80]
    F000 = s0 * (wb[0][:, None] * WL0[1][0:16]) @ WM1[1][64:80]
    WB = np.zeros((128, 64))
    WB[0:64] = WM1[1][0:64]
    WB[64:80] = F110
    WB[80:96] = F220
    WB[96:112] = F110
    WB[112:128] = F220
    W["WB"] = f(WB)                                           # (128,64)
    W["WV"] = f(F000)                                         # (16,64)
    W["bm1b"] = f(bM1[1].reshape(S, 1))
    W["Wpm2b"] = f(WM2[1])                                    # (64,64)
    W["bm2a"] = f(bM2[0].reshape(S, 1))
    W["bm2b"] = f(bM2[1].reshape(S, 1))
    return W


def _pack_weights(W):
    names = list(W.keys())
    offs = {}
    col = 0
    for nm in names:
        k, m = W[nm].shape
        offs[nm] = (k, m, col)
        col += m
    arr = np.zeros((128, col), BF)
    for nm in names:
        k, m, o = offs[nm]
        arr[:k, o:o + m] = W[nm].astype(BF)
    return arr, offs


def _build_nc(woffs, wcols):
    nc = bacc.Bacc()
    h_p = nc.declare_dram_parameter("h", [32, EC], BF16, isOutput=False)
    g_p = nc.declare_dram_parameter("geo", [112, EC], BF16, isOutput=False)
    wpack_p = nc.declare_dram_parameter("wpack", [128, wcols], BF16, isOutput=False)
    b32_p = nc.declare_dram_parameter("b32", [128, 4], F32, isOutput=False)
    out_p = nc.declare_dram_parameter("out", [NL, S, EC], BF16, isOutput=True)

    h_ap = h_p[:]
    g_ap = g_p[:]
    out_ap = out_p[:]

    PF = 4  # dma prefetch depth (iterations)
    AOP = mybir.AluOpType

    with TileContext(nc) as tc:
        with (
            tc.tile_pool(name="const", bufs=1) as constp,
            tc.tile_pool(name="work", bufs=8) as work,
            tc.tile_pool(name="ps1", bufs=1, space="PSUM") as ps1,
            tc.tile_pool(name="ps2", bufs=2, space="PSUM") as ps2,
        ):
            wpack = constp.tile([128, wcols], BF16, name="wpack", tag="wpack")
            nc.sync.dma_start(out=wpack, in_=wpack_p[:])
            b32 = constp.tile([128, 4], F32, name="b32", tag="b32")
            nc.sync.dma_start(out=b32, in_=b32_p[:])

            class _WT:
                def __getitem__(self, nm):
                    k, m, o = woffs[nm]
                    return wpack[:k, o:o + m]

            wt = _WT()

            def wslice(nm, k0, k1, m0, m1):
                k, m, o = woffs[nm]
                return wpack[k0:k1, o + m0:o + m1]

            # PE warm-up burst once weights land (flips HAM to 8/8).
            warm = ps2.tile([128, N], F32, name="warm", tag="pmf")
            nc.tensor.matmul(warm[:1, :1], wpack[:1, :1], wpack[:1, :1],
                             start=True, stop=True)
            for _ in range(10):
                nc.tensor.matmul(warm[:64, :N], wpack[:128, :64],
                                 wpack[:128, :N], start=True, stop=True)

            st = {}

            def dma_in(b):
                st.setdefault(b, {})
                HS = work.tile([96, N], BF16, name="HS", tag="HS")
                st[b]["HS"] = HS
                if b < NBLK:
                    o = b * N
                    nc.sync.dma_start(out=HS[64:96], in_=h_ap[:, slice(o, o + N)])
                    GT = work.tile([112, N], BF16, name="GT", tag="GT")
                    st[b]["GT"] = GT
                    nc.sync.dma_start(out=GT, in_=g_ap[:, slice(o, o + N)])

            def s1(i):
                # M12: [pre-sb1(i) 0:64 (from hT(i)); pe2(i-1) 64:128 (from
                # sb1(i-1) at HS(i)[32:96])]
                d = st[i]
                PX2 = ps1.tile([128, N], F32, name="PX2", tag="PX2")
                if i == 0:
                    nc.tensor.matmul(PX2[0:64], wslice("WM12", 64, 96, 0, 64),
                                     d["HS"][64:96], start=True, stop=True)
                elif i == NBLK:
                    nc.tensor.matmul(PX2[64:128], wslice("WM12", 0, 64, 64, 128),
                                     d["HS"][0:64], start=True, stop=True)
                else:
                    nc.tensor.matmul(PX2[:128], wt["WM12"], d["HS"][0:96],
                                     start=True, stop=True)
                if i < NBLK:
                    nc.scalar.activation(st[i + 1]["HS"][0:64], PX2[0:64],
                                         ACT.Silu, bias=wt["be1"])
                if i > 0:
                    dp = st[i - 1]
                    PMT = work.tile([80, N], BF16, name="PMT", tag="PMT")
                    dp["PMT"] = PMT
                    nc.scalar.activation(PMT[0:64], PX2[64:128], ACT.Silu,
                                         bias=wt["be2"])

            def s2a(b):
                d = st[b]
                PMT = d["PMT"]
                pxe = ps1.tile([112, N], F32, name="pxe", tag="pxe")
                nc.tensor.matmul(pxe[64:112], wt["Wenv0"], PMT[0:64],
                                 start=True, stop=True)
                EQ = work.tile([48, N], BF16, name="EQ", tag="EQ")
                nc.vector.scalar_tensor_tensor(
                    EQ, pxe[64:112], b32[64:112, 2:3], d["GT"][64:112],
                    AOP.add, AOP.mult)
                pA = ps1.tile([80, N], F32, name="pA", tag="pA")
                nc.tensor.matmul(pA[:80], wt["WA"], EQ, start=True, stop=True)
                U80 = work.tile([80, N], BF16, name="U80", tag="U80")
                d["U80"] = U80
                nc.vector.tensor_mul(U80, pA[0:80], d["GT"][0:80])
                nc.scalar.activation(PMT[64:80], pA[64:80], ACT.Copy)

            def s2b(b):
                d = st[b]
                pm1 = ps1.tile([65, N], F32, name="pm1", tag="pm1")
                nc.tensor.matmul(pm1[:65], wt["Wpm1"], d["PMT"][0:80],
                                 start=True, stop=True)
                mh0 = work.tile([65, N], BF16, name="mh0", tag="mh0")
                d["mh0"] = mh0
                nc.scalar.activation(mh0, pm1[:65], ACT.Silu, bias=wt["bm1a"])

            def s3(b):
                d = st[b]
                pmf = ps2.tile([128, N], F32, name="pmf", tag="pmf")
                d["pmf"] = pmf
                nc.tensor.matmul(pmf[:128], wt["Wpm2e"], d["mh0"],
                                 start=True, stop=True)
                CT = work.tile([128, N], BF16, name="CT", tag="CT")
                d["CT"] = CT
                nc.scalar.activation(CT[0:64], pmf[0:64], ACT.Identity,
                                     bias=wt["bm2a"])
                nc.vector.tensor_mul(CT[64:128], d["U80"][0:64], pmf[64:128])
                VT = work.tile([16, N], BF16, name="VT", tag="VT")
                d["VT"] = VT
                nc.vector.tensor_mul(VT, d["U80"][64:80], pmf[64:80])

            def s4(b):
                d = st[b]
                pm1b = ps1.tile([64, N], F32, name="pm1b", tag="pm1b")
                nc.tensor.matmul(pm1b[:64], wt["WB"], d["CT"],
                                 start=True, stop=False)
                nc.tensor.matmul(pm1b[:64], wt["WV"], d["VT"],
                                 start=False, stop=True)
                mh1 = work.tile([64, N], BF16, name="mh1", tag="mh1")
                d["mh1"] = mh1
                nc.scalar.activation(mh1, pm1b[:64], ACT.Silu, bias=wt["bm1b"])

            def s5(b):
                d = st[b]
                pxe = ps1.tile([112, N], F32, name="pxe", tag="pxe")
                nc.tensor.matmul(pxe[0:64], wt["Wpm2b"], d["mh1"],
                                 start=True, stop=True)
                CT = d["CT"]
                nc.vector.tensor_scalar_add(CT[64:128], pxe[0:64],
                                            b32[0:64, 1:2])
                o = b * N
                dst = bass.AP(tensor=out_ap.tensor, offset=o,
                              ap=[[EC, 128], [1, N]])
                nc.sync.dma_start(out=dst, in_=CT[0:128])
                del st[b]

            for b in range(min(PF, NBLK + 1)):
                dma_in(b)
            for i in range(NBLK + 7):
                if i <= NBLK:
                    s1(i)
                if 0 <= i - 2 < NBLK:
                    s2a(i - 2)
                if 0 <= i - 3 < NBLK:
                    s2b(i - 3)
                if 0 <= i - 4 < NBLK:
                    s3(i - 4)
                if 0 <= i - 5 < NBLK:
                    s4(i - 5)
                if 0 <= i - 6 < NBLK:
                    s5(i - 6)
                if i + PF <= NBLK:
                    dma_in(i + PF)
    nc.finalize()
    return nc


_NC_CACHE = None


def _host_prep(inputs):
    bond_dist = np.asarray(inputs["bond_dist"], np.float32)
    bond_diff = np.asarray(inputs["bond_diff"], np.float32)
    emb = np.asarray(inputs["emb_table"], np.float32)
    Z = np.asarray(inputs["Z"]).astype(np.int64)
    ei = np.asarray(inputs["edge_index"]).astype(np.int64)

    u = bond_dist / RMAX
    n = np.arange(1, NB + 1, dtype=np.float32)
    radial = (np.sqrt(np.float32(2.0 / RMAX)) *
              np.sin(np.float32(np.pi) * n * u[:, None].astype(np.float32)) /
              bond_dist[:, None])
    cutoff = np.where(u < 1.0, 1.0 - 28.0 * u**6 + 48.0 * u**7 - 21.0 * u**8, 0.0)
    radial = (radial * cutoff[:, None].astype(np.float32)).astype(np.float32)

    d = (bond_diff / (bond_dist[:, None] + np.float32(1e-8))).astype(np.float32)
    Qf = _Q.astype(np.float32)
    y2 = (np.sqrt(np.float32(1.5)) *
          np.einsum('mij,ei,ej->em', Qf, d, d)).astype(np.float32)
    ss = np.einsum('ei,ei->e', d, d)
    s4 = np.einsum('em,em->e', y2, y2)
    v = np.einsum('em,mij,ej->ei', y2, Qf, d)
    vd = np.einsum('ei,ei->e', v, d)
    a = np.einsum('mpq,ep,eq->em', _A.astype(np.float32), y2, y2)
    ay = np.einsum('em,em->e', a, y2)

    te = (emb[Z[ei[:, 0]]] * emb[Z[ei[:, 1]]]).astype(np.float32)
    h = np.zeros((32, E), np.float32)
    h[0:24] = np.concatenate([radial, te], axis=1).T
    h = np.ascontiguousarray(h.astype(BF))

    # rows 0:80 feed U80 = pA * [s2,s4,vd,ay,1]; rows 64:112 = [1,s2,s4]
    # feed EQ (the ones-section is shared between both windows).
    geo = np.zeros((112, E), np.float32)
    geo[0:16] = ss
    geo[16:32] = s4
    geo[32:48] = vd
    geo[48:64] = ay
    geo[64:80] = 1.0
    geo[80:96] = ss
    geo[96:112] = s4
    geo = np.ascontiguousarray(geo.astype(BF))

    W = _fold_weights(inputs)
    b32 = np.zeros((128, 4), np.float32)
    b32[0:64, 0] = W["bm2a"][:, 0]
    b32[0:64, 1] = W["bm2b"][:, 0]
    benv_e = np.asarray(inputs["b_env_e"], np.float32)
    b32[64:112, 2] = np.concatenate([benv_e] * 3)
    return h, geo, W, b32


def make_in_maps(inputs):
    global _NC_CACHE
    h, geo, W, b32 = _host_prep(inputs)
    wpack, woffs = _pack_weights(W)
    if _NC_CACHE is None:
        _NC_CACHE = _build_nc(woffs, wpack.shape[1])
    in_maps = []
    for i in range(NCORES):
        sl = slice(i * EC, (i + 1) * EC)
        m = {"h": np.ascontiguousarray(h[:, sl]),
             "geo": np.ascontiguousarray(geo[:, sl]),
             "wpack": wpack, "b32": b32}
        in_maps.append(m)
    return in_maps


def kernel(**inputs):
    in_maps = make_in_maps(inputs)
    res = run_bass_kernel_spmd(_NC_CACHE, in_maps, list(range(NCORES))).results
    out = np.concatenate(
        [np.asarray(res[i]["out"]).astype(np.float32).transpose(2, 0, 1)
         for i in range(NCORES)], axis=0)
    return np.ascontiguousarray(out)


# revision 35
# speedup vs baseline: 3.7279x; 1.1420x over previous
"""Allegro-style equivariant GNN edge-network on 8 TRN2 NeuronCores — v4.

Key identity: at layer 0, x0/x1/x2 = (y-irrep) * env[c], so the full O3
tensor-product + equivariant-linear stack collapses to per-edge *scalar*
geometry features (s2=|d|^2, s4=|d|^4, vd=v.d, ay=a.y2 — host precomputed)
combined with 16x16 folded weight matmuls on env:
  n1[i,c] = d_i*(A1@env)[c] + v_i*(A2@env)[c]
  n2[m,c] = y2_m*(B1@env)[c] + a_m*(B2@env)[c]
  n0[c]   = (C1@env + s2*C2@env + s4*C3@env)[c]
Layer-1 needs only the invariant channel -> 8 matmuls, 4 silus, 5 DVE ops,
3 gpsimd ops per 512-edge block.

v4: 5-stage software pipeline across blocks (interleaved emission keeps all
engine queues fed), one PSUM bank per tile kind (pe1/pm2b share a bank),
single fused out-DMA, no memsets (pad rows produced as silu(-20)~=0).
"""

import sys

sys.path.insert(0, "/opt/trn_rl_repo")

import numpy as np
import ml_dtypes

BF = ml_dtypes.bfloat16

import concourse.bass as bass
import concourse.mybir as mybir
from concourse import bacc
from concourse.tile import TileContext
from concourse.bass_utils import run_bass_kernel_spmd

E = 131072
NCORES = 8
EC = E // NCORES
C = 16
S = 64
NB = 8
TE = 16
NL = 2
RMAX = 5.0

N = 512
NBLK = EC // N

F32 = mybir.dt.float32
BF16 = mybir.dt.bfloat16
ACT = mybir.ActivationFunctionType


def _Qnp():
    Q = np.zeros((5, 3, 3))
    s = 1.0 / np.sqrt(2.0)
    Q[0, 0, 1] = Q[0, 1, 0] = s
    Q[1, 1, 2] = Q[1, 2, 1] = s
    Q[2] = np.diag([-1.0, -1.0, 2.0]) / np.sqrt(6.0)
    Q[3, 0, 2] = Q[3, 2, 0] = s
    Q[4] = np.diag([1.0, -1.0, 0.0]) * s
    return Q


_Q = _Qnp()
_An = np.einsum('mij,pjk,qki->mpq', _Q, _Q, _Q)
_A = 0.5 * (_An + _An.transpose(0, 2, 1))


def _fold_weights(inp):
    f = lambda a: np.ascontiguousarray(a, dtype=np.float32)
    W = {}
    s0 = 1.0 / np.sqrt(3.0 * C)
    s1 = 1.0 / np.sqrt(4.0 * C)
    s2c = 1.0 / np.sqrt(4.0 * C)

    We1 = np.asarray(inp["W_e1"], np.float64)
    be1 = np.asarray(inp["b_e1"], np.float64)
    We2 = np.asarray(inp["W_e2"], np.float64)
    be2 = np.asarray(inp["b_e2"], np.float64)
    Wenv_e = np.asarray(inp["W_env_e"], np.float64)
    benv_e = np.asarray(inp["b_env_e"], np.float64)
    Wenv = np.asarray(inp["Wenv"], np.float64)
    benv = np.asarray(inp["benv"], np.float64)
    WM1 = np.asarray(inp["Wm1"], np.float64)
    bM1 = np.asarray(inp["bm1"], np.float64)
    WM2 = np.asarray(inp["Wm2"], np.float64)
    bM2 = np.asarray(inp["bm2"], np.float64)
    wtp = np.asarray(inp["w_tp"], np.float64)
    WL0 = np.asarray(inp["Wlin0"], np.float64)
    WL1 = np.asarray(inp["Wlin1"], np.float64)
    WL2 = np.asarray(inp["Wlin2"], np.float64)

    _bias1 = float(np.float32(1.2784645427610783).astype(BF))
    ones_val = _bias1 / (1.0 + np.exp(-_bias1))  # exact bf16-rounded silu(bias1)

    # merged edge-MLP matmul: rhs = HS = [hT(i) 0:24; sb1(i-1) 32:96],
    # out = [pre-sb1(i) 0:64; pe2(i-1) 64:128]
    WM12 = np.zeros((96, 128))
    WM12[0:64, 64:128] = We2
    WM12[64:88, 0:64] = We1
    W["WM12"] = f(WM12)                                       # (96,128)
    W["be1"] = f(be1.reshape(S, 1))
    W["be2"] = f(be2.reshape(S, 1))

    # merged env0(b) + pm2b(b-4): rhs = MC = [mh1(b-4) 0:64; comb0(b) 64:128]
    WM39 = np.zeros((128, 112))
    WM39[0:64, 0:64] = WM2[1]
    WM39[64:128, 64:112] = np.hstack([Wenv_e] * 3)
    W["WM39"] = f(WM39)                                       # (128,112)

    # collapsed layer-0 TP: WA maps EQ48 -> [P1 P3 P2 P4 n0]
    w = wtp[0]
    A1 = s1 * (w[1][:, None] * WL1[0][0:16] + w[3][:, None] * WL1[0][16:32])
    A2 = s1 * (w[6][:, None] * WL1[0][32:48] + w[8][:, None] * WL1[0][48:64])
    B1 = s2c * (w[2][:, None] * WL2[0][0:16]
                + (w[5] / np.sqrt(1.5))[:, None] * WL2[0][16:32]
                + w[7][:, None] * WL2[0][32:48])
    B2 = s2c * (w[10][:, None] * WL2[0][48:64])
    C1 = s0 * (w[0][:, None] * WL0[0][0:16])
    C2 = s0 * (w[4][:, None] * WL0[0][16:32])
    C3 = s0 * (w[9][:, None] * WL0[0][32:48])
    WA = np.zeros((48, 80))
    WA[0:16, 0:16] = A1
    WA[0:16, 16:32] = B1
    WA[0:16, 32:48] = A2
    WA[0:16, 48:64] = B2
    WA[0:16, 64:80] = C1
    WA[16:32, 64:80] = C2
    WA[32:48, 64:80] = C3
    W["WA"] = f(WA)                                           # (48,80)

    # pm1: rhs = PMT[0:112] = [comb0 0:64; EQ 64:112]; the n0 path is
    # folded through EQ: n0 = Cmat@EQ, so Wm1i@n0 = (Cmat@Wm1i)@EQ.
    Cmat = np.concatenate([C1, C2, C3], axis=0)               # (48,16)
    CW = Cmat @ WM1[0][64:80]                                 # (48,64)
    Wpm1 = np.zeros((112, 65))
    Wpm1[0:64, 0:64] = WM1[0][0:64]
    Wpm1[64:112, 0:64] = CW
    W["Wpm1"] = f(Wpm1)                                       # (112,65)
    W["bm1a"] = f(np.vstack([bM1[0].reshape(S, 1), [[_bias1]]]))

    # pm2 + env1 fused: rhs = mh0[0:65]; out rows 0:64 = pm2 (scal1 pre-bias),
    # rows 64:128 = env1 x4 copies (bias via ones-row 64 of mh0)
    Wpm2e = np.zeros((65, 128))
    Wpm2e[0:64, 0:64] = WM2[0]
    Wpm2e[0:64, 64:128] = np.hstack([WM2[0] @ Wenv[0]] * 4)
    Wpm2e[64, 64:128] = np.hstack(
        [(bM2[0] @ Wenv[0] + benv[0]).reshape(1, C)] * 4)[0] / ones_val
    W["Wpm2e"] = f(Wpm2e)                                     # (65,128)

    # pm1b: rhs = CT = [comb1 0:64; V64 = (s2P1,s4P3,vdP2,ayP4)*env1 64:128]
    # pair-sums (s2P1+vdP2 -> t110, s4P3+ayP4 -> t220) folded into repeated
    # F-blocks; t000 (n0*env1) comes in via the separate VT matmul.
    wb = wtp[1]
    F110 = s0 * (wb[4][:, None] * WL0[1][16:32]) @ WM1[1][64:80]
    F220 = s0 * (wb[9][:, None] * WL0[1][32:48]) @ WM1[1][64:80]
    F000 = s0 * (wb[0][:, None] * WL0[1][0:16]) @ WM1[1][64:80]
    WB = np.zeros((128, 64))
    WB[0:64] = WM1[1][0:64]
    WB[64:80] = F110
    WB[80:96] = F220
    WB[96:112] = F110
    WB[112:128] = F220
    W["WB"] = f(WB)                                           # (128,64)
    W["WV"] = f(F000)                                         # (16,64)
    W["bm1b"] = f(bM1[1].reshape(S, 1))
    W["bm2a"] = f(bM2[0].reshape(S, 1))
    W["bm2b"] = f(bM2[1].reshape(S, 1))
    return W


_ROWOFF = {"WA": 64}


def _pack_weights(W):
    names = list(W.keys())
    offs = {}
    col = 0
    for nm in names:
        k, m = W[nm].shape
        offs[nm] = (k, m, col, _ROWOFF.get(nm, 0))
        col += m
    arr = np.zeros((128, col), BF)
    for nm in names:
        k, m, o, r = offs[nm]
        arr[r:r + k, o:o + m] = W[nm].astype(BF)
    return arr, offs


def _build_nc(woffs, wcols):
    nc = bacc.Bacc()
    h_p = nc.declare_dram_parameter("h", [32, EC], BF16, isOutput=False)
    g_p = nc.declare_dram_parameter("geo", [112, EC], BF16, isOutput=False)
    wpack_p = nc.declare_dram_parameter("wpack", [128, wcols], BF16, isOutput=False)
    b32_p = nc.declare_dram_parameter("b32", [128, 4], F32, isOutput=False)
    out_p = nc.declare_dram_parameter("out", [NL, S, EC], BF16, isOutput=True)

    h_ap = h_p[:]
    g_ap = g_p[:]
    out_ap = out_p[:]

    PF = 4  # dma prefetch depth (iterations)
    AOP = mybir.AluOpType

    with TileContext(nc) as tc:
        with (
            tc.tile_pool(name="const", bufs=1) as constp,
            tc.tile_pool(name="work", bufs=8) as work,
            tc.tile_pool(name="ps1", bufs=1, space="PSUM") as ps1,
            tc.tile_pool(name="ps2", bufs=2, space="PSUM") as ps2,
        ):
            wpack = constp.tile([128, wcols], BF16, name="wpack", tag="wpack")
            nc.sync.dma_start(out=wpack, in_=wpack_p[:])
            b32 = constp.tile([128, 4], F32, name="b32", tag="b32")
            nc.sync.dma_start(out=b32, in_=b32_p[:])

            class _WT:
                def __getitem__(self, nm):
                    k, m, o, r = woffs[nm]
                    return wpack[r:r + k, o:o + m]

            wt = _WT()

            def wslice(nm, k0, k1, m0, m1):
                k, m, o, r = woffs[nm]
                return wpack[r + k0:r + k1, o + m0:o + m1]

            # PE warm-up burst once weights land (flips HAM to 8/8).
            warm = ps2.tile([128, N], F32, name="warm", tag="pmf")
            nc.tensor.matmul(warm[:1, :1], wpack[:1, :1], wpack[:1, :1],
                             start=True, stop=True)
            for _ in range(10):
                nc.tensor.matmul(warm[:64, :N], wpack[:128, :64],
                                 wpack[:128, :N], start=True, stop=True)

            st = {}
            mc = {}

            def dma_in(b):
                st.setdefault(b, {})
                HS = work.tile([96, N], BF16, name="HS", tag="HS")
                st[b]["HS"] = HS
                if b < NBLK:
                    o = b * N
                    nc.sync.dma_start(out=HS[64:96], in_=h_ap[:, slice(o, o + N)])
                    GT = work.tile([112, N], BF16, name="GT", tag="GT")
                    st[b]["GT"] = GT
                    nc.sync.dma_start(out=GT, in_=g_ap[:, slice(o, o + N)])

            def s1(i):
                # M12: [pre-sb1(i) 0:64 (from hT(i) at HS[64:96]);
                #       pe2(i-1) 64:128 (from sb1(i-1) at HS[0:64])]
                d = st[i]
                PX2 = ps2.tile([128, N], F32, name="PX2", tag="PX2")
                if i == 0:
                    nc.tensor.matmul(PX2[0:64], wslice("WM12", 64, 96, 0, 64),
                                     d["HS"][64:96], start=True, stop=True)
                elif i == NBLK:
                    nc.tensor.matmul(PX2[64:128], wslice("WM12", 0, 64, 64, 128),
                                     d["HS"][0:64], start=True, stop=True)
                else:
                    nc.tensor.matmul(PX2[:128], wt["WM12"], d["HS"][0:96],
                                     start=True, stop=True)
                if i < NBLK:
                    nc.scalar.activation(st[i + 1]["HS"][0:64], PX2[0:64],
                                         ACT.Silu, bias=wt["be1"])
                if i > 0:
                    dp = st[i - 1]
                    PMT = work.tile([112, N], BF16, name="PMT", tag="PMT")
                    dp["PMT"] = PMT
                    nc.scalar.activation(PMT[0:64], PX2[64:128], ACT.Silu,
                                         bias=wt["be2"])
                    nc.gpsimd.tensor_copy(mc[i][64:128], PMT[0:64])

            def s39(b):
                # merged env0(b) [out rows 64:112] + pm2b(b-4) [out rows 0:64]
                # rhs = mc[b+1] = [mh1(b-4) 0:64; comb0(b) 64:128]
                pxe = ps1.tile([112, N], F32, name="pxe", tag="pxe")
                st.setdefault(b, {})["pxe"] = pxe
                has_m3 = b < NBLK
                has_m9 = 0 <= b - 4 < NBLK
                r = mc[b + 1]
                if has_m3 and has_m9:
                    nc.tensor.matmul(pxe[0:112], wt["WM39"], r[0:128],
                                     start=True, stop=True)
                elif has_m3:
                    nc.tensor.matmul(pxe[64:112], wslice("WM39", 64, 128, 64, 112),
                                     r[64:128], start=True, stop=True)
                elif has_m9:
                    nc.tensor.matmul(pxe[0:64], wslice("WM39", 0, 64, 0, 64),
                                     r[0:64], start=True, stop=True)

            def s2a(b):
                d = st[b]
                PMT = d["PMT"]
                pxe = d["pxe"]
                nc.vector.scalar_tensor_tensor(
                    PMT[64:112], pxe[64:112], b32[64:112, 2:3],
                    d["GT"][64:112], AOP.add, AOP.mult)
                pA = ps1.tile([80, N], F32, name="pA", tag="pA")
                nc.tensor.matmul(pA[:80], wt["WA"], PMT[64:112],
                                 start=True, stop=True)
                U80 = work.tile([80, N], BF16, name="U80", tag="U80")
                d["U80"] = U80
                nc.vector.tensor_mul(U80, pA[0:80], d["GT"][0:80])

            def s2b(b):
                d = st[b]
                pm1 = ps1.tile([65, N], F32, name="pm1", tag="pm1")
                nc.tensor.matmul(pm1[:65], wt["Wpm1"], d["PMT"][0:112],
                                 start=True, stop=True)
                mh0 = work.tile([65, N], BF16, name="mh0", tag="mh0")
                d["mh0"] = mh0
                nc.scalar.activation(mh0, pm1[:65], ACT.Silu, bias=wt["bm1a"])

            def s3(b):
                d = st[b]
                pmf = ps2.tile([128, N], F32, name="pmf", tag="pmf")
                nc.tensor.matmul(pmf[:128], wt["Wpm2e"], d["mh0"],
                                 start=True, stop=True)
                CT = work.tile([128, N], BF16, name="CT", tag="CT")
                d["CT"] = CT
                nc.scalar.activation(CT[0:64], pmf[0:64], ACT.Identity,
                                     bias=wt["bm2a"])
                nc.vector.tensor_mul(CT[64:128], d["U80"][0:64], pmf[64:128])
                VT = work.tile([16, N], BF16, name="VT", tag="VT")
                d["VT"] = VT
                nc.vector.tensor_mul(VT, d["U80"][64:80], pmf[64:80])

            def s4(b):
                d = st[b]
                pm1b = ps1.tile([64, N], F32, name="pm1b", tag="pm1b")
                nc.tensor.matmul(pm1b[:64], wt["WB"], d["CT"],
                                 start=True, stop=False)
                nc.tensor.matmul(pm1b[:64], wt["WV"], d["VT"],
                                 start=False, stop=True)
                nc.scalar.activation(mc[b + 5][0:64], pm1b[:64], ACT.Silu,
                                     bias=wt["bm1b"])

            def s5(b):
                d = st[b]
                CT = d["CT"]
                pxe = st[b + 4]["pxe"]
                nc.vector.tensor_scalar_add(CT[64:128], pxe[0:64],
                                            b32[0:64, 1:2])
                o = b * N
                dst = bass.AP(tensor=out_ap.tensor, offset=o,
                              ap=[[EC, 128], [1, N]])
                nc.gpsimd.dma_start(out=dst, in_=CT[0:128])
                del st[b]

            for b in range(min(PF, NBLK + 1)):
                dma_in(b)
            for i in range(NBLK + 7):
                mc[i] = work.tile([128, N], BF16, name="MC", tag="MC")
                if i <= NBLK:
                    s1(i)
                if 0 <= i - 2 < NBLK + 4:
                    s39(i - 2)
                if 0 <= i - 2 < NBLK:
                    s2a(i - 2)
                if 0 <= i - 3 < NBLK:
                    s2b(i - 3)
                if 0 <= i - 4 < NBLK:
                    s3(i - 4)
                if 0 <= i - 5 < NBLK:
                    s4(i - 5)
                if 0 <= i - 6 < NBLK:
                    s5(i - 6)
                if i + PF <= NBLK:
                    dma_in(i + PF)
    nc.finalize()
    return nc


_NC_CACHE = None


def _host_prep(inputs):
    bond_dist = np.asarray(inputs["bond_dist"], np.float32)
    bond_diff = np.asarray(inputs["bond_diff"], np.float32)
    emb = np.asarray(inputs["emb_table"], np.float32)
    Z = np.asarray(inputs["Z"]).astype(np.int64)
    ei = np.asarray(inputs["edge_index"]).astype(np.int64)

    u = bond_dist / RMAX
    n = np.arange(1, NB + 1, dtype=np.float32)
    radial = (np.sqrt(np.float32(2.0 / RMAX)) *
              np.sin(np.float32(np.pi) * n * u[:, None].astype(np.float32)) /
              bond_dist[:, None])
    cutoff = np.where(u < 1.0, 1.0 - 28.0 * u**6 + 48.0 * u**7 - 21.0 * u**8, 0.0)
    radial = (radial * cutoff[:, None].astype(np.float32)).astype(np.float32)

    d = (bond_diff / (bond_dist[:, None] + np.float32(1e-8))).astype(np.float32)
    Qf = _Q.astype(np.float32)
    y2 = (np.sqrt(np.float32(1.5)) *
          np.einsum('mij,ei,ej->em', Qf, d, d)).astype(np.float32)
    ss = np.einsum('ei,ei->e', d, d)
    s4 = np.einsum('em,em->e', y2, y2)
    v = np.einsum('em,mij,ej->ei', y2, Qf, d)
    vd = np.einsum('ei,ei->e', v, d)
    a = np.einsum('mpq,ep,eq->em', _A.astype(np.float32), y2, y2)
    ay = np.einsum('em,em->e', a, y2)

    te = (emb[Z[ei[:, 0]]] * emb[Z[ei[:, 1]]]).astype(np.float32)
    h = np.zeros((32, E), np.float32)
    h[0:24] = np.concatenate([radial, te], axis=1).T
    h = np.ascontiguousarray(h.astype(BF))

    # rows 0:80 feed U80 = pA * [s2,s4,vd,ay,1]; rows 64:112 = [1,s2,s4]
    # feed EQ (the ones-section is shared between both windows).
    geo = np.zeros((112, E), np.float32)
    geo[0:16] = ss
    geo[16:32] = s4
    geo[32:48] = vd
    geo[48:64] = ay
    geo[64:80] = 1.0
    geo[80:96] = ss
    geo[96:112] = s4
    geo = np.ascontiguousarray(geo.astype(BF))

    W = _fold_weights(inputs)
    b32 = np.zeros((128, 4), np.float32)
    b32[0:64, 0] = W["bm2a"][:, 0]
    b32[0:64, 1] = W["bm2b"][:, 0]
    benv_e = np.asarray(inputs["b_env_e"], np.float32)
    b32[64:112, 2] = np.concatenate([benv_e] * 3)
    return h, geo, W, b32


def make_in_maps(inputs):
    global _NC_CACHE
    h, geo, W, b32 = _host_prep(inputs)
    wpack, woffs = _pack_weights(W)
    if _NC_CACHE is None:
        _NC_CACHE = _build_nc(woffs, wpack.shape[1])
    in_maps = []
    for i in range(NCORES):
        sl = slice(i * EC, (i + 1) * EC)
        m = {"h": np.ascontiguousarray(h[:, sl]),
             "geo": np.ascontiguousarray(geo[:, sl]),
             "wpack": wpack, "b32": b32}
        in_maps.append(m)
    return in_maps


def kernel(**inputs):
    in_maps = make_in_maps(inputs)
    res = run_bass_kernel_spmd(_NC_CACHE, in_maps, list(range(NCORES))).results
    out = np.concatenate(
        [np.asarray(res[i]["out"]).astype(np.float32).transpose(2, 0, 1)
         for i in range(NCORES)], axis=0)
    return np.ascontiguousarray(out)
